# revision 2
# baseline (speedup 1.0000x reference)
"""Trainium2 Bass kernel v3 for the PCNN bag-classification model.

Design:
  - Balanced bag-boundary sharding over 8 cores (no collectives).
  - Host ships the full fp8 DR-interleaved conv input stream (no on-device
    gather): channels 0..255 in the main stream tile, the 44 leftover word
    channels + 10 positional channels (pre-shifted per tap) + the mask
    channel in a packed tile.
  - Block-aligned piece layout: each sentence occupies 128 slots = 32 blocks
    of 4; the three PCNN pieces are padded to block boundaries.  Pad slots
    are killed by a conv mask channel (-8); real piece1/2 slots carry -4 so
    block maxes of foreign pieces always lose.
  - conv1d(k=3) as 4 DoubleRow fp8 matmuls per (subgroup=4 sentences,
    filter-chunk), weights batched across 3 subgroups to amortize LDWEIGHTS.
  - Hierarchical max-pool: scalar engine copies PSUM->SBUF f16 slabs, DVE
    reduces blocks of 4 (stage 1) into a per-core block-max array; per group
    of 8 subgroups, DVE computes the 3 phase maxima over block maxes with
    {-4,0,+4} block deltas (stage 2, 8x less data than slot level).
  - Block deltas are broadcast across partitions with a ones-matmul on the
    PE (instead of a 128x DMA broadcast).
  - Dense + bag-mean (segment mean as matmul with per-bag 1/count weights) +
    softmax on-chip, pipelined per 128-sentence chunk.
"""

import os
import sys

for _p in ("/opt/trn_rl_repo",):
    if _p not in sys.path:
        sys.path.insert(0, _p)

import numpy as np
import ml_dtypes

# ---------------- problem constants ----------------
N = 2048
L = 120
NCORES = 8
NF = 230
NREL = 53
NBAGS = 256
VOCAB = 100000
WD = 300
PD = 5

BLK = 8              # slots per block
SBLK = 17            # blocks per sentence
SLOTS = BLK * SBLK   # 136 slots per sentence
SPSG = 3             # sentences per subgroup (PSUM bank: 408 <= 512 f32)
SGW = SPSG * SLOTS   # 408 slots per subgroup
BPSG = SPSG * SBLK   # 51 blocks per subgroup
GRPW = 8 * SGW       # 3264 slots per group (8 subgroups)
PSW = 512            # PSUM tile free width (per filter chunk)
MB = 4.0
FCH = [(0, 128, 128), (128, 102, 112)]  # (f0, fw_real, fw_pad)

FP8 = ml_dtypes.float8_e4m3
F16 = np.float16

_PROGRAM_CACHE = {}
_LAYOUT_CACHE = {}
LAST_RESULT = None


# ---------------- per-sentence block layout ----------------
def _sentence_layout(lo, hi):
    """Returns (srcpos[128] int64 into the 122-wide edge-padded arrays,
    slotmask[128] float {0,-MB,-2MB}, blockpiece[32] int64)."""
    key = (lo, hi)
    hit = _LAYOUT_CACHE.get(key)
    if hit is not None:
        return hit
    lens = [lo, hi - lo, L - hi]
    starts = [0, lo, hi]
    B0 = -(-lens[0] // BLK)
    B1 = -(-lens[1] // BLK)
    B2 = SBLK - B0 - B1
    assert B2 * BLK >= lens[2], (lo, hi)
    Bs = [B0, B1, B2]
    p = [Bs[i] * BLK - lens[i] for i in range(3)]
    sol = None
    for f0 in range(p[0] + 1):
        for f1 in range(p[1] + 1):
            for f2 in range(p[2] + 1):
                b0, b1, b2 = p[0] - f0, p[1] - f1, p[2] - f2
                if (b0 + f1) != 1 and (b1 + f2) != 1 and b2 >= 2:
                    sol = (f0, f1, f2)
                    break
            if sol:
                break
        if sol:
            break
    assert sol is not None, (lens, p)
    f = sol
    srcpos = np.zeros(SLOTS, np.int64)
    slotmask = np.full(SLOTS, -2 * MB, np.float32)
    blockpiece = np.zeros(SBLK, np.int64)
    s = 0
    bidx = 0
    rs, re = [], []
    for i in range(3):
        a, ln = starts[i], lens[i]
        blockpiece[bidx:bidx + Bs[i]] = i
        bidx += Bs[i]
        s += f[i]
        rs.append(s)
        srcpos[s:s + ln] = np.arange(a + 1, a + ln + 1)
        slotmask[s:s + ln] = 0.0 if i == 0 else -MB
        s += ln
        re.append(s)
        s += p[i] - f[i]
    assert s == SLOTS
    srcpos[0:rs[0]] = 0
    for i in range(2):
        r0, r1 = re[i], rs[i + 1]
        if r1 > r0:
            srcpos[r0:r1] = starts[i] + lens[i] + 1
            srcpos[r1 - 1] = starts[i + 1]
    srcpos[re[2]:SLOTS] = L + 1
    out = (srcpos, slotmask, blockpiece)
    _LAYOUT_CACHE[key] = out
    return out


# ---------------- device program ----------------
def _build_program(nsg, ngrp, bags_cap, nchunk):
    import concourse.bass as bass
    import concourse.mybir as mybir
    import concourse.tile as tile
    from concourse import bacc

    f32 = mybir.dt.float32
    f16 = mybir.dt.float16
    fp8 = mybir.dt.float8e4
    AF = mybir.ActivationFunctionType
    AX = mybir.AxisListType
    ALU = mybir.AluOpType
    DR = mybir.MatmulPerfMode.DoubleRow

    ns_pad = SPSG * nsg
    nblk = ns_pad * SBLK            # total blocks per core
    dcols = 2 * nblk                # delta row columns (2 phases)
    nd = (dcols + 511) // 512       # delta broadcast chunks
    TW = 2 * (GRPW + 2)             # stream tile bytes per partition
    GP_ON = bool(int(os.environ.get("KERNEL_GP", "1")))

    nc = bacc.Bacc(
        "TRN2", target_bir_lowering=False, debug=False, num_devices=NCORES,
        num_swdge_queues=4,
    )

    gt_d = nc.dram_tensor("gt8", [ngrp, 128, TW], fp8, kind="ExternalInput").ap()
    pk_d = nc.dram_tensor("pk8", [ngrp, 112, TW], fp8, kind="ExternalInput").ap()
    wdr_d = nc.dram_tensor("wdr", [3, 128, 2 * 240], fp8,
                           kind="ExternalInput").ap()
    wp_d = nc.dram_tensor("wp", [112, 2 * 240], fp8, kind="ExternalInput").ap()
    drow_d = nc.dram_tensor("drow", [1, nd * 512], f16,
                            kind="ExternalInput").ap()
    dwt_d = nc.dram_tensor("dwt", [128, 6 * NREL], f16,
                           kind="ExternalInput").ap()
    actb_d = nc.dram_tensor("actb", [128, 2], f32, kind="ExternalInput").ap()
    dbias_d = nc.dram_tensor("dbias", [1, NREL], f16, kind="ExternalInput").ap()
    snorm_d = nc.dram_tensor("snorm", [nchunk * 128, bags_cap], f16,
                             kind="ExternalInput").ap()
    out_d = nc.dram_tensor("out", [bags_cap, NREL], f32,
                           kind="ExternalOutput").ap()

    with tile.TileContext(nc) as tc:
        import contextlib

        ctx = contextlib.ExitStack()
        with ctx:
            singles = ctx.enter_context(tc.tile_pool(name="singles", bufs=1))

            wdr_sb = [singles.tile([128, 2, 240], fp8, name=f"wdr{t}")
                      for t in range(3)]
            wp_sb = singles.tile([112, 2, 240], fp8)
            dwt_sb = singles.tile([128, 6 * NREL], f16)
            actb_sb = singles.tile([128, 2], f32)
            dbias_sb = singles.tile([1, NREL], f16)
            snorm_sb = [singles.tile([128, bags_cap], f16, name=f"sn{c}")
                        for c in range(nchunk)]
            ones_sb = singles.tile([1, 128], f16)
            ball = [singles.tile([128, nblk], f16, name=f"ball{c}")
                    for c in range(2)]
            delta_sb = singles.tile([128, 2, nblk], f16)
            scr = [singles.tile([128, 8 * BPSG], f16, name=f"scr{c}")
                   for c in range(2)]
            gtmp = singles.tile([128, 3, 256], f32, name="gtmp")
            pooled = [singles.tile([128, 3, ns_pad], f16, name=f"pool{c}")
                      for c in range(2)]
            pr = [singles.tile([128, 3, ns_pad], f16, name=f"pr{c}")
                  for c in range(2)]
            lgs = [singles.tile([128, NREL], f16, name=f"lgs{c}")
                   for c in range(nchunk)]

            gt_pool = ctx.enter_context(tc.tile_pool(name="gt", bufs=3))
            pk_pool = ctx.enter_context(tc.tile_pool(name="pk", bufs=3))
            cp_pool = ctx.enter_context(tc.tile_pool(name="cp", bufs=4))
            cv_psum = ctx.enter_context(
                tc.tile_pool(name="cv", bufs=3, space="PSUM"))
            t_psum = ctx.enter_context(
                tc.tile_pool(name="tp", bufs=2, space="PSUM"))

            gt_tiles = {}
            pk_tiles = {}

            def fetch_group(g):
                if g in gt_tiles or g >= ngrp:
                    return
                gt = gt_pool.tile([128, TW], fp8, tag="gt", name=f"gt{g}")
                nc.sync.dma_start(out=gt[:, :], in_=gt_d[g, :, :])
                pk = pk_pool.tile([112, TW], fp8, tag="pk", name=f"pk{g}")
                nc.sync.dma_start(out=pk[:, :], in_=pk_d[g, :, :])
                gt_tiles[g] = gt
                pk_tiles[g] = pk

            # conv inputs first so the PE can start ASAP
            for t in range(3):
                nc.sync.dma_start(out=wdr_sb[t][:, :, :], in_=wdr_d[t, :, :])
            nc.sync.dma_start(out=wp_sb[:, :, :], in_=wp_d[:, :])
            fetch_group(0)
            fetch_group(1)
            nc.sync.dma_start(out=dwt_sb[:, :], in_=dwt_d[:, :])
            nc.sync.dma_start(out=actb_sb[:, :], in_=actb_d[:, :])
            nc.sync.dma_start(out=dbias_sb[:, :], in_=dbias_d[:, :])
            for c in range(nchunk):
                nc.sync.dma_start(out=snorm_sb[c][:, :],
                                  in_=snorm_d[c * 128:(c + 1) * 128, :])
            nc.vector.memset(ones_sb[:, :], 1.0)

            # ---- broadcast the block deltas across partitions via DMA ----
            dview = delta_sb[:, 0, 0]
            dflat = bass.AP(tensor=dview.tensor, offset=dview.offset,
                            ap=[dview.ap[0], [1, 2 * nblk]])
            nc.sync.dma_start(
                out=dflat,
                in_=drow_d[:, 0:2 * nblk].to_broadcast((128, 2 * nblk)))

            BATCH = 3
            lgs_done = [False] * nchunk

            def stage2_group(g):
                """Phase maxima for group g's 32 sentences (DVE)."""
                sg0 = 8 * g
                sgn = min(8, nsg - sg0)
                b0 = sg0 * BPSG         # first block index
                nb = sgn * BPSG         # blocks in this group
                s0 = SPSG * sg0         # first sentence
                nsent = SPSG * sgn
                for fc in range(2):
                    fw = FCH[fc][1]
                    bsl = ball[fc][0:fw, b0:b0 + nb]
                    bs3 = bass.AP(tensor=bsl.tensor, offset=bsl.offset,
                                  ap=[bsl.ap[0], [SBLK, nsent], [1, SBLK]])
                    nc.vector.tensor_reduce(
                        out=pooled[fc][0:fw, 0, s0:s0 + nsent],
                        in_=bs3, axis=AX.X, op=ALU.max)
                    teng = nc.gpsimd if (GP_ON and fc == 1) else nc.vector
                    for ph in range(2):
                        sc = scr[fc][0:fw, 0:nb]
                        teng.tensor_tensor(
                            out=sc, in0=bsl,
                            in1=delta_sb[0:fw, ph, b0:b0 + nb],
                            op=ALU.add)
                        sc3 = bass.AP(tensor=sc.tensor, offset=sc.offset,
                                      ap=[sc.ap[0], [SBLK, nsent], [1, SBLK]])
                        nc.vector.tensor_reduce(
                            out=pooled[fc][0:fw, 1 + ph, s0:s0 + nsent],
                            in_=sc3, axis=AX.X, op=ALU.max)

            def finish_chunk(c):
                """relu + logits matmul for sentence chunk c (128 sentences)."""
                if lgs_done[c]:
                    return
                lgs_done[c] = True
                cs = min(128, ns_pad - 128 * c)
                for fc in range(2):
                    fw = FCH[fc][1]
                    nc.scalar.activation(
                        out=pr[fc][0:fw, :, 128 * c:128 * c + cs],
                        in_=pooled[fc][0:fw, :, 128 * c:128 * c + cs],
                        func=AF.Relu, bias=actb_sb[0:fw, fc:fc + 1], scale=1.0)
                lg_ps = t_psum.tile([128, NREL], f32, tag="tp", name=f"lg{c}")
                nmm = 0
                for j in range(3):
                    for fc, (f0, fw, fwp) in enumerate(FCH):
                        nc.tensor.matmul(
                            out=lg_ps[0:cs, :],
                            lhsT=pr[fc][0:fw, j, 128 * c:128 * c + cs],
                            rhs=dwt_sb[0:fw, (j * 2 + fc) * NREL:
                                       (j * 2 + fc + 1) * NREL],
                            start=(nmm == 0), stop=False,
                            skip_group_check=True)
                        nmm += 1
                nc.tensor.matmul(
                    out=lg_ps[0:cs, :],
                    lhsT=ones_sb[0:1, 0:cs],
                    rhs=dbias_sb[0:1, :],
                    start=False, stop=True, skip_group_check=True)
                nc.scalar.copy(out=lgs[c][0:cs, :], in_=lg_ps[0:cs, :])

            kk = 0
            while kk < nsg:
                bn = min(BATCH, nsg - kk)
                # prefetch a future group
                fetch_group(kk // 8 + 2)
                ps = []
                slab = cp_pool.tile([128, bn, 2, PSW], f16, tag="cp",
                                    name=f"sl{kk}")
                for i in range(bn):
                    ps.append(cv_psum.tile([128, 2, PSW], f32, tag="cv",
                                           name=f"cv{kk + i}"))
                # conv matmuls, weights-major for LDW amortization
                for fc, (f0, fw, fwp) in enumerate(FCH):
                    for t in range(4):
                        for i in range(bn):
                            g = (kk + i) // 8
                            l = (kk + i) % 8
                            gtt = gt_tiles[g]
                            if t < 3:
                                gb = gtt[:, 0]
                                rhs = bass.AP(
                                    tensor=gb.tensor,
                                    offset=gb.offset + 2 * (SGW * l + t),
                                    ap=[gb.ap[0], [1, 2], [2, SGW]])
                                nc.tensor.matmul(
                                    out=ps[i][0:fwp, fc, 0:SGW],
                                    lhsT=wdr_sb[t][:, :, f0:f0 + fwp],
                                    rhs=rhs, start=(t == 0), stop=False,
                                    perf_mode=DR, skip_group_check=True)
                            else:
                                pkt = pk_tiles[g]
                                pb = pkt[0:112, 0]
                                rhsp = bass.AP(
                                    tensor=pb.tensor,
                                    offset=pb.offset + 2 * (SGW * l + 1),
                                    ap=[[pb.ap[0][0], 112], [1, 2], [2, SGW]])
                                nc.tensor.matmul(
                                    out=ps[i][0:fwp, fc, 0:SGW],
                                    lhsT=wp_sb[0:112, :, f0:f0 + fwp],
                                    rhs=rhsp, start=False, stop=True,
                                    perf_mode=DR, skip_group_check=True)
                # PSUM -> SBUF f16 slab (scalar engine), both fc at once
                for i in range(bn):
                    nc.scalar.copy(out=slab[:, i, :, 0:SGW],
                                   in_=ps[i][:, :, 0:SGW])
                # stage 1: block maxes (DVE)
                for fc, (f0, fw, fwp) in enumerate(FCH):
                    sl = slab[0:fw, 0, 0, 0]
                    sl4 = bass.AP(tensor=sl.tensor,
                                  offset=sl.offset + PSW * fc,
                                  ap=[sl.ap[0], [2 * PSW, bn], [BLK, BPSG],
                                      [1, BLK]])
                    nc.vector.tensor_reduce(
                        out=ball[fc][0:fw, kk * BPSG:(kk + bn) * BPSG],
                        in_=sl4, axis=AX.X, op=ALU.max)
                kk += bn
                # group boundary: run stage 2 for completed groups
                gdone = kk // 8
                gprev = (kk - bn) // 8
                for g in range(gprev, min(gdone, ngrp)):
                    if 8 * (g + 1) <= kk or kk == nsg:
                        stage2_group(g)
                        # sentence chunks fully covered by finished groups
                        sdone = min(8 * (g + 1), nsg) * SPSG
                        for c in range(nchunk):
                            if (c + 1) * 128 <= sdone:
                                finish_chunk(c)
            if nsg % 8 != 0:
                stage2_group(nsg // 8)
            for c in range(nchunk):
                finish_chunk(c)

            # ---- bag mean + softmax ----
            bg_ps = t_psum.tile([128, NREL], f32, tag="tp", name="bg")
            for c in range(nchunk):
                cs = min(128, ns_pad - 128 * c)
                nc.tensor.matmul(
                    out=bg_ps[0:bags_cap, :],
                    lhsT=snorm_sb[c][0:cs, :],
                    rhs=lgs[c][0:cs, :],
                    start=(c == 0), stop=(c == nchunk - 1),
                    skip_group_check=True)

            t = singles.tile([128, NREL], f32, name="sm")
            nc.vector.tensor_copy(out=t[0:bags_cap, :],
                                  in_=bg_ps[0:bags_cap, :])
            nmax = singles.tile([128, 1], f32, name="nmax")
            nc.vector.reduce_max(out=nmax[0:bags_cap, :], in_=t[0:bags_cap, :],
                                 axis=AX.X, negate=True)
            ex = singles.tile([128, NREL], f32, name="ex")
            nc.scalar.activation(out=ex[0:bags_cap, :], in_=t[0:bags_cap, :],
                                 func=AF.Exp, bias=nmax[0:bags_cap, :],
                                 scale=1.0)
            ssum = singles.tile([128, 1], f32, name="ssum")
            nc.vector.reduce_sum(out=ssum[0:bags_cap, :],
                                 in_=ex[0:bags_cap, :], axis=AX.X)
            rcp = singles.tile([128, 1], f32, name="rcp")
            nc.vector.reciprocal(out=rcp[0:bags_cap, :],
                                 in_=ssum[0:bags_cap, :])
            res = singles.tile([128, NREL], f32, name="res")
            nc.vector.tensor_scalar_mul(res[0:bags_cap, :],
                                        ex[0:bags_cap, :],
                                        rcp[0:bags_cap, :])
            nc.sync.dma_start(out=out_d[:, :], in_=res[0:bags_cap, :])

    nc.compile()
    return nc


def _pad_edge(a):
    return np.concatenate([a[:, :1], a, a[:, -1:]], axis=1)


def kernel(**inputs):
    global LAST_RESULT
    sentences = np.asarray(inputs["sentences"]).astype(np.int64)
    pos1 = np.asarray(inputs["pos1"]).astype(np.int64)
    pos2 = np.asarray(inputs["pos2"]).astype(np.int64)
    masks = np.asarray(inputs["masks"]).astype(np.float32)
    bag_ids = np.asarray(inputs["bag_ids"]).astype(np.int64)
    word_emb = np.asarray(inputs["word_emb"]).astype(np.float32)
    pf1_emb = np.asarray(inputs["pf1_emb"]).astype(np.float32)
    pf2_emb = np.asarray(inputs["pf2_emb"]).astype(np.float32)
    conv_w = np.asarray(inputs["conv_w"]).astype(np.float32)
    conv_b = np.asarray(inputs["conv_b"]).astype(np.float32)
    dense_w = np.asarray(inputs["dense_w"]).astype(np.float32)
    dense_b = np.asarray(inputs["dense_b"]).astype(np.float32)

    # ---- balanced bag-boundary sharding ----
    counts = np.bincount(bag_ids, minlength=NBAGS)
    cum = np.concatenate([[0], np.cumsum(counts)])
    B = [0]
    for r in range(1, NCORES):
        B.append(int(np.argmin(np.abs(cum - N * r // NCORES))))
    B.append(NBAGS)
    for r in range(1, NCORES + 1):
        B[r] = max(B[r], B[r - 1])
    S = [int(cum[b]) for b in B]
    cnt = [S[r + 1] - S[r] for r in range(NCORES)]
    ncap = max(max(cnt), 1)
    nsg = (ncap + SPSG - 1) // SPSG
    ns_pad = SPSG * nsg
    ngrp = (nsg + 7) // 8
    bags_cap = max(B[r + 1] - B[r] for r in range(NCORES))
    nchunk = (ns_pad + 127) // 128
    nblk = ns_pad * SBLK
    dcols = 2 * nblk
    nd = (dcols + 511) // 512
    TW = 2 * (GRPW + 2)

    key = (nsg, ngrp, bags_cap, nchunk)
    if key not in _PROGRAM_CACHE:
        _PROGRAM_CACHE[key] = _build_program(nsg, ngrp, bags_cap, nchunk)
    nc = _PROGRAM_CACHE[key]

    # ---- shared parameter prep ----
    e8 = word_emb.astype(FP8)                    # [V, 300]
    e8main = np.ascontiguousarray(e8[:, :256])   # [V, 256]
    e8left = np.zeros((VOCAB, 44), FP8)
    e8left[:, :] = e8[:, 256:300]
    pf1_8 = pf1_emb.astype(FP8)                  # [240, 5]
    pf2_8 = pf2_emb.astype(FP8)

    wdr = np.zeros((3, 128, 2, 240), np.float32)
    for t in range(3):
        for i in range(2):
            wdr[t, :, i, :NF] = conv_w[:, i:256:2, t].T
    wdr = wdr.astype(FP8).reshape(3, 128, 480)

    wp = np.zeros((112, 2, 240), np.float32)
    for t in range(3):
        for i in range(2):
            wp[32 * t:32 * t + 22, i, :NF] = conv_w[:, 256 + i:300:2, t].T
            wp[96 + 5 * t:96 + 5 * t + 5, i, :NF] = conv_w[:, 300 + i:310:2, t].T
    wp[111, 0, :NF] = 1.0  # mask channel rides the center tap
    wp = wp.astype(FP8).reshape(112, 480)

    dwt = np.zeros((128, 6 * NREL), np.float32)
    for j in range(3):
        for fc, (f0, fw, fwp) in enumerate(FCH):
            dwt[:fw, (j * 2 + fc) * NREL:(j * 2 + fc + 1) * NREL] = \
                dense_w[:, j * NF + f0:j * NF + f0 + fw].T
    dwt = dwt.astype(F16)

    actb = np.zeros((128, 2), np.float32)
    for fc, (f0, fw, fwp) in enumerate(FCH):
        actb[:fw, fc] = conv_b[f0:f0 + fw]

    dbias = dense_b.reshape(1, NREL).astype(F16)
    fcounts = np.maximum(counts.astype(np.float32), 1.0)

    piece_all = masks.argmax(axis=1)                      # [N, 120]
    lo_all = (piece_all >= 1).argmax(axis=1)
    hi_all = (piece_all >= 2).argmax(axis=1)

    in_maps = []
    for r in range(NCORES):
        s0r, s1r = S[r], S[r + 1]
        nreal = s1r - s0r
        sent = np.zeros((ns_pad, L), np.int64)
        sent[:nreal] = sentences[s0r:s1r]
        p1 = np.zeros((ns_pad, L), np.int64)
        p1[:nreal] = pos1[s0r:s1r]
        p2 = np.zeros((ns_pad, L), np.int64)
        p2[:nreal] = pos2[s0r:s1r]

        sp = _pad_edge(sent)    # [ns_pad, 122]
        p1p = _pad_edge(p1)
        p2p = _pad_edge(p2)

        srcpos = np.zeros((ns_pad, SLOTS), np.int64)
        slotmask = np.full((ns_pad, SLOTS), -2 * MB, np.float32)
        bp = np.zeros((ns_pad, SBLK), np.int64)
        for i in range(nreal):
            a, b_, c_ = _sentence_layout(int(lo_all[s0r + i]),
                                         int(hi_all[s0r + i]))
            srcpos[i], slotmask[i], bp[i] = a, b_, c_

        tok = np.take_along_axis(sp, srcpos, axis=1)      # [ns_pad, 128]
        p1s = np.take_along_axis(p1p, srcpos, axis=1)
        p2s = np.take_along_axis(p2p, srcpos, axis=1)
        # cross-sentence fixup: last trailing slot carries the next
        # sentence's left-edge column
        tok[:-1, -1] = sp[1:, 0]
        p1s[:-1, -1] = p1p[1:, 0]
        p2s[:-1, -1] = p2p[1:, 0]

        Stot = ns_pad * SLOTS
        ghal_t = np.empty(Stot + 4, np.int64)
        ghal_t[2:-2] = tok.reshape(-1)
        ghal_t[:2] = sp[0, 0]
        ghal_t[-2:] = ghal_t[-3]
        ghal_1 = np.empty(Stot + 4, np.int64)
        ghal_1[2:-2] = p1s.reshape(-1)
        ghal_1[:2] = p1p[0, 0]
        ghal_1[-2:] = ghal_1[-3]
        ghal_2 = np.empty(Stot + 4, np.int64)
        ghal_2[2:-2] = p2s.reshape(-1)
        ghal_2[:2] = p2p[0, 0]
        ghal_2[-2:] = ghal_2[-3]
        ghal_m = np.full(Stot + 4, -2 * MB, np.float32)
        ghal_m[2:-2] = slotmask.reshape(-1)
        m8 = ghal_m.astype(FP8).view(np.uint8)

        gt8 = np.zeros((ngrp, 128, TW), np.uint8)
        pk8 = np.zeros((ngrp, 112, TW), np.uint8)
        for g in range(ngrp):
            u0 = GRPW * g + 1           # ghal index of tile u=0 (slot -1)
            idx = np.arange(u0, u0 + GRPW + 2)
            idx = np.minimum(idx, Stot + 3)
            arr = e8main[ghal_t[idx]].view(np.uint16)       # [4098, 128]
            gt8[g] = np.ascontiguousarray(arr.T).view(np.uint8).reshape(
                128, TW)
            for t in range(3):
                it = np.clip(idx + (t - 1), 0, Stot + 3)
                lv = e8left[ghal_t[it]].view(np.uint16)     # [4098, 22]
                pk8[g, 32 * t:32 * t + 22] = np.ascontiguousarray(
                    lv.T).view(np.uint8).reshape(22, TW)
                pfv = np.concatenate(
                    [pf1_8[ghal_1[it]], pf2_8[ghal_2[it]]],
                    axis=1).view(np.uint16)                  # [4098, 5]
                pk8[g, 96 + 5 * t:96 + 5 * t + 5] = np.ascontiguousarray(
                    pfv.T).view(np.uint8).reshape(5, TW)
            pk8[g, 111, 0::2] = m8[idx]
        gt8 = gt8.view(FP8)
        pk8 = pk8.view(FP8)

        # block deltas: d1 then d2, fp8 row
        drow = np.zeros((1, nd * 512), np.float32)
        d1 = np.where(bp == 1, MB, np.where(bp == 0, -MB, 0.0))
        d2 = np.where(bp == 2, MB, np.where(bp == 0, -MB, 0.0))
        drow[0, :nblk] = d1.reshape(-1)
        drow[0, nblk:2 * nblk] = d2.reshape(-1)
        drow = drow.astype(F16)

        snorm = np.zeros((nchunk * 128, bags_cap), np.float32)
        bags = bag_ids[s0r:s1r]
        snorm[np.arange(nreal), bags - B[r]] = 1.0 / fcounts[bags]
        snorm = snorm.astype(F16)

        in_maps.append({
            "gt8": gt8,
            "pk8": pk8,
            "wdr": wdr,
            "wp": wp,
            "drow": drow,
            "dwt": dwt,
            "actb": actb,
            "dbias": dbias,
            "snorm": snorm,
        })

    from concourse.bass_utils import run_bass_kernel_spmd

    trace = bool(int(os.environ.get("KERNEL_TRACE", "0")))
    res = run_bass_kernel_spmd(
        nc, in_maps, core_ids=list(range(NCORES)), trace=trace
    )
    LAST_RESULT = res

    out = np.zeros((NBAGS, NREL), np.float32)
    for r in range(NCORES):
        nb = B[r + 1] - B[r]
        if nb > 0:
            out[B[r]:B[r + 1]] = res.results[r]["out"][:nb].astype(np.float32)
    return out


if __name__ == "__main__":
    d = np.load("/root/problem/ref_inputs.npz")
    out = kernel(**{k: d[k] for k in d.files})
    print("out", out.shape, out.dtype)


# revision 3
# speedup vs baseline: 1.0189x; 1.0189x over previous
"""Trainium2 Bass kernel v3 for the PCNN bag-classification model.

Design:
  - Balanced bag-boundary sharding over 8 cores (no collectives).
  - Host ships the full fp8 DR-interleaved conv input stream (no on-device
    gather): channels 0..255 in the main stream tile, the 44 leftover word
    channels + 10 positional channels (pre-shifted per tap) + the mask
    channel in a packed tile.
  - Block-aligned piece layout: each sentence occupies 128 slots = 32 blocks
    of 4; the three PCNN pieces are padded to block boundaries.  Pad slots
    are killed by a conv mask channel (-8); real piece1/2 slots carry -4 so
    block maxes of foreign pieces always lose.
  - conv1d(k=3) as 4 DoubleRow fp8 matmuls per (subgroup=4 sentences,
    filter-chunk), weights batched across 3 subgroups to amortize LDWEIGHTS.
  - Hierarchical max-pool: scalar engine copies PSUM->SBUF f16 slabs, DVE
    reduces blocks of 4 (stage 1) into a per-core block-max array; per group
    of 8 subgroups, DVE computes the 3 phase maxima over block maxes with
    {-4,0,+4} block deltas (stage 2, 8x less data than slot level).
  - Block deltas are broadcast across partitions with a ones-matmul on the
    PE (instead of a 128x DMA broadcast).
  - Dense + bag-mean (segment mean as matmul with per-bag 1/count weights) +
    softmax on-chip, pipelined per 128-sentence chunk.
"""

import os
import sys

for _p in ("/opt/trn_rl_repo",):
    if _p not in sys.path:
        sys.path.insert(0, _p)

import numpy as np
import ml_dtypes

# ---------------- problem constants ----------------
N = 2048
L = 120
NCORES = 8
NF = 230
NREL = 53
NBAGS = 256
VOCAB = 100000
WD = 300
PD = 5

BLK = 8              # slots per block
SBLK = 17            # blocks per sentence
SLOTS = BLK * SBLK   # 136 slots per sentence
SPSG = 3             # sentences per subgroup (PSUM bank: 408 <= 512 f32)
SGW = SPSG * SLOTS   # 408 slots per subgroup
BPSG = SPSG * SBLK   # 51 blocks per subgroup
GRPW = 8 * SGW       # 3264 slots per group (8 subgroups)
HG = 4               # subgroups per fetch tile (half group)
HGW = HG * SGW       # 1632 slots per fetch tile
PSW = 512            # PSUM tile free width (per filter chunk)
MB = 4.0
FCH = [(0, 128, 128), (128, 102, 112)]  # (f0, fw_real, fw_pad)

FP8 = ml_dtypes.float8_e4m3
F16 = np.float16

_PROGRAM_CACHE = {}
_LAYOUT_CACHE = {}
LAST_RESULT = None


# ---------------- per-sentence block layout ----------------
def _sentence_layout(lo, hi):
    """Returns (srcpos[128] int64 into the 122-wide edge-padded arrays,
    slotmask[128] float {0,-MB,-2MB}, blockpiece[32] int64)."""
    key = (lo, hi)
    hit = _LAYOUT_CACHE.get(key)
    if hit is not None:
        return hit
    lens = [lo, hi - lo, L - hi]
    starts = [0, lo, hi]
    B0 = -(-lens[0] // BLK)
    B1 = -(-lens[1] // BLK)
    B2 = SBLK - B0 - B1
    assert B2 * BLK >= lens[2], (lo, hi)
    Bs = [B0, B1, B2]
    p = [Bs[i] * BLK - lens[i] for i in range(3)]
    sol = None
    for f0 in range(p[0] + 1):
        for f1 in range(p[1] + 1):
            for f2 in range(p[2] + 1):
                b0, b1, b2 = p[0] - f0, p[1] - f1, p[2] - f2
                if (b0 + f1) != 1 and (b1 + f2) != 1 and b2 >= 2:
                    sol = (f0, f1, f2)
                    break
            if sol:
                break
        if sol:
            break
    assert sol is not None, (lens, p)
    f = sol
    srcpos = np.zeros(SLOTS, np.int64)
    slotmask = np.full(SLOTS, -2 * MB, np.float32)
    blockpiece = np.zeros(SBLK, np.int64)
    s = 0
    bidx = 0
    rs, re = [], []
    for i in range(3):
        a, ln = starts[i], lens[i]
        blockpiece[bidx:bidx + Bs[i]] = i
        bidx += Bs[i]
        s += f[i]
        rs.append(s)
        srcpos[s:s + ln] = np.arange(a + 1, a + ln + 1)
        slotmask[s:s + ln] = 0.0 if i == 0 else -MB
        s += ln
        re.append(s)
        s += p[i] - f[i]
    assert s == SLOTS
    srcpos[0:rs[0]] = 0
    for i in range(2):
        r0, r1 = re[i], rs[i + 1]
        if r1 > r0:
            srcpos[r0:r1] = starts[i] + lens[i] + 1
            srcpos[r1 - 1] = starts[i + 1]
    srcpos[re[2]:SLOTS] = L + 1
    out = (srcpos, slotmask, blockpiece)
    _LAYOUT_CACHE[key] = out
    return out


# ---------------- device program ----------------
def _build_program(nsg, ngrp, bags_cap, nchunk):
    import concourse.bass as bass
    import concourse.mybir as mybir
    import concourse.tile as tile
    from concourse import bacc

    f32 = mybir.dt.float32
    f16 = mybir.dt.float16
    fp8 = mybir.dt.float8e4
    AF = mybir.ActivationFunctionType
    AX = mybir.AxisListType
    ALU = mybir.AluOpType
    DR = mybir.MatmulPerfMode.DoubleRow

    ns_pad = SPSG * nsg
    nblk = ns_pad * SBLK            # total blocks per core
    dcols = 2 * nblk                # delta row columns (2 phases)
    nd = (dcols + 511) // 512       # delta broadcast chunks
    TW = 2 * (HGW + 2)              # stream tile bytes per partition
    GP_ON = bool(int(os.environ.get("KERNEL_GP", "1")))

    nc = bacc.Bacc(
        "TRN2", target_bir_lowering=False, debug=False, num_devices=NCORES,
        num_swdge_queues=1,
    )

    gt_d = nc.dram_tensor("gt8", [ngrp, 128, TW], fp8, kind="ExternalInput").ap()
    pk_d = nc.dram_tensor("pk8", [ngrp, 112, TW], fp8, kind="ExternalInput").ap()
    wdr_d = nc.dram_tensor("wdr", [3, 128, 2 * 240], fp8,
                           kind="ExternalInput").ap()
    wp_d = nc.dram_tensor("wp", [112, 2 * 240], fp8, kind="ExternalInput").ap()
    drow_d = nc.dram_tensor("drow", [1, nd * 512], f16,
                            kind="ExternalInput").ap()
    dwt_d = nc.dram_tensor("dwt", [128, 6 * NREL], f16,
                           kind="ExternalInput").ap()
    actb_d = nc.dram_tensor("actb", [128, 2], f32, kind="ExternalInput").ap()
    dbias_d = nc.dram_tensor("dbias", [1, NREL], f16, kind="ExternalInput").ap()
    snorm_d = nc.dram_tensor("snorm", [nchunk * 128, bags_cap], f16,
                             kind="ExternalInput").ap()
    out_d = nc.dram_tensor("out", [bags_cap, NREL], f32,
                           kind="ExternalOutput").ap()

    with tile.TileContext(nc) as tc:
        import contextlib

        ctx = contextlib.ExitStack()
        with ctx:
            singles = ctx.enter_context(tc.tile_pool(name="singles", bufs=1))

            wdr_sb = [singles.tile([128, 2, 240], fp8, name=f"wdr{t}")
                      for t in range(3)]
            wp_sb = singles.tile([112, 2, 240], fp8)
            dwt_sb = singles.tile([128, 6 * NREL], f16)
            actb_sb = singles.tile([128, 2], f32)
            dbias_sb = singles.tile([1, NREL], f16)
            snorm_sb = [singles.tile([128, bags_cap], f16, name=f"sn{c}")
                        for c in range(nchunk)]
            ones_sb = singles.tile([1, 128], f16)
            ball = [singles.tile([128, nblk], f16, name=f"ball{c}")
                    for c in range(2)]
            delta_sb = singles.tile([128, 2, nblk], f16)
            scr = [singles.tile([128, 8 * BPSG], f16, name=f"scr{c}")
                   for c in range(2)]
            gtmp = singles.tile([128, 3, 256], f32, name="gtmp")
            pooled = [singles.tile([128, 3, ns_pad], f16, name=f"pool{c}")
                      for c in range(2)]
            pr = [singles.tile([128, 3, ns_pad], f16, name=f"pr{c}")
                  for c in range(2)]
            lgs = [singles.tile([128, NREL], f16, name=f"lgs{c}")
                   for c in range(nchunk)]

            gt_pool = ctx.enter_context(tc.tile_pool(name="gt", bufs=4))
            pk_pool = ctx.enter_context(tc.tile_pool(name="pk", bufs=4))
            cp_pool = ctx.enter_context(tc.tile_pool(name="cp", bufs=4))
            cv_psum = ctx.enter_context(
                tc.tile_pool(name="cv", bufs=3, space="PSUM"))
            t_psum = ctx.enter_context(
                tc.tile_pool(name="tp", bufs=2, space="PSUM"))

            gt_tiles = {}
            pk_tiles = {}

            def fetch_group(g):
                if g in gt_tiles or g >= ngrp:
                    return
                gt = gt_pool.tile([128, TW], fp8, tag="gt", name=f"gt{g}")
                nc.sync.dma_start(out=gt[:, :], in_=gt_d[g, :, :])
                pk = pk_pool.tile([112, TW], fp8, tag="pk", name=f"pk{g}")
                nc.sync.dma_start(out=pk[:, :], in_=pk_d[g, :, :])
                gt_tiles[g] = gt
                pk_tiles[g] = pk

            # conv inputs first so the PE can start ASAP
            for t in range(3):
                nc.sync.dma_start(out=wdr_sb[t][:, :, :], in_=wdr_d[t, :, :])
            nc.sync.dma_start(out=wp_sb[:, :, :], in_=wp_d[:, :])
            fetch_group(0)
            fetch_group(1)
            fetch_group(2)
            nc.sync.dma_start(out=dwt_sb[:, :], in_=dwt_d[:, :])
            nc.sync.dma_start(out=actb_sb[:, :], in_=actb_d[:, :])
            nc.sync.dma_start(out=dbias_sb[:, :], in_=dbias_d[:, :])
            for c in range(nchunk):
                nc.sync.dma_start(out=snorm_sb[c][:, :],
                                  in_=snorm_d[c * 128:(c + 1) * 128, :])
            nc.vector.memset(ones_sb[:, :], 1.0)

            # ---- broadcast the block deltas across partitions via DMA ----
            dview = delta_sb[:, 0, 0]
            dflat = bass.AP(tensor=dview.tensor, offset=dview.offset,
                            ap=[dview.ap[0], [1, 2 * nblk]])
            nc.sync.dma_start(
                out=dflat,
                in_=drow_d[:, 0:2 * nblk].to_broadcast((128, 2 * nblk)))

            BATCH = 3
            lgs_done = [False] * nchunk

            def stage2_group(g):
                """Phase maxima for group g's 32 sentences (DVE)."""
                sg0 = 8 * g
                sgn = min(8, nsg - sg0)
                b0 = sg0 * BPSG         # first block index
                nb = sgn * BPSG         # blocks in this group
                s0 = SPSG * sg0         # first sentence
                nsent = SPSG * sgn
                for fc in range(2):
                    fw = FCH[fc][1]
                    bsl = ball[fc][0:fw, b0:b0 + nb]
                    bs3 = bass.AP(tensor=bsl.tensor, offset=bsl.offset,
                                  ap=[bsl.ap[0], [SBLK, nsent], [1, SBLK]])
                    nc.vector.tensor_reduce(
                        out=pooled[fc][0:fw, 0, s0:s0 + nsent],
                        in_=bs3, axis=AX.X, op=ALU.max)
                    teng = nc.gpsimd if (GP_ON and fc == 1) else nc.vector
                    for ph in range(2):
                        sc = scr[fc][0:fw, 0:nb]
                        teng.tensor_tensor(
                            out=sc, in0=bsl,
                            in1=delta_sb[0:fw, ph, b0:b0 + nb],
                            op=ALU.add)
                        sc3 = bass.AP(tensor=sc.tensor, offset=sc.offset,
                                      ap=[sc.ap[0], [SBLK, nsent], [1, SBLK]])
                        nc.vector.tensor_reduce(
                            out=pooled[fc][0:fw, 1 + ph, s0:s0 + nsent],
                            in_=sc3, axis=AX.X, op=ALU.max)

            def finish_chunk(c):
                """relu + logits matmul for sentence chunk c (128 sentences)."""
                if lgs_done[c]:
                    return
                lgs_done[c] = True
                cs = min(128, ns_pad - 128 * c)
                for fc in range(2):
                    fw = FCH[fc][1]
                    nc.scalar.activation(
                        out=pr[fc][0:fw, :, 128 * c:128 * c + cs],
                        in_=pooled[fc][0:fw, :, 128 * c:128 * c + cs],
                        func=AF.Relu, bias=actb_sb[0:fw, fc:fc + 1], scale=1.0)
                lg_ps = t_psum.tile([128, NREL], f32, tag="tp", name=f"lg{c}")
                nmm = 0
                for j in range(3):
                    for fc, (f0, fw, fwp) in enumerate(FCH):
                        nc.tensor.matmul(
                            out=lg_ps[0:cs, :],
                            lhsT=pr[fc][0:fw, j, 128 * c:128 * c + cs],
                            rhs=dwt_sb[0:fw, (j * 2 + fc) * NREL:
                                       (j * 2 + fc + 1) * NREL],
                            start=(nmm == 0), stop=False,
                            skip_group_check=True)
                        nmm += 1
                nc.tensor.matmul(
                    out=lg_ps[0:cs, :],
                    lhsT=ones_sb[0:1, 0:cs],
                    rhs=dbias_sb[0:1, :],
                    start=False, stop=True, skip_group_check=True)
                nc.scalar.copy(out=lgs[c][0:cs, :], in_=lg_ps[0:cs, :])

            kk = 0
            while kk < nsg:
                bn = min(BATCH, nsg - kk)
                # prefetch a future fetch tile
                fetch_group(kk // HG + 3)
                ps = []
                slab = cp_pool.tile([128, bn, 2, PSW], f16, tag="cp",
                                    name=f"sl{kk}")
                for i in range(bn):
                    ps.append(cv_psum.tile([128, 2, PSW], f32, tag="cv",
                                           name=f"cv{kk + i}"))
                # conv matmuls, weights-major for LDW amortization
                for fc, (f0, fw, fwp) in enumerate(FCH):
                    for t in range(4):
                        for i in range(bn):
                            g = (kk + i) // HG
                            l = (kk + i) % HG
                            gtt = gt_tiles[g]
                            if t < 3:
                                gb = gtt[:, 0]
                                rhs = bass.AP(
                                    tensor=gb.tensor,
                                    offset=gb.offset + 2 * (SGW * l + t),
                                    ap=[gb.ap[0], [1, 2], [2, SGW]])
                                nc.tensor.matmul(
                                    out=ps[i][0:fwp, fc, 0:SGW],
                                    lhsT=wdr_sb[t][:, :, f0:f0 + fwp],
                                    rhs=rhs, start=(t == 0), stop=False,
                                    perf_mode=DR, skip_group_check=True)
                            else:
                                pkt = pk_tiles[g]
                                pb = pkt[0:112, 0]
                                rhsp = bass.AP(
                                    tensor=pb.tensor,
                                    offset=pb.offset + 2 * (SGW * l + 1),
                                    ap=[[pb.ap[0][0], 112], [1, 2], [2, SGW]])
                                nc.tensor.matmul(
                                    out=ps[i][0:fwp, fc, 0:SGW],
                                    lhsT=wp_sb[0:112, :, f0:f0 + fwp],
                                    rhs=rhsp, start=False, stop=True,
                                    perf_mode=DR, skip_group_check=True)
                # PSUM -> SBUF f16 slab (scalar engine), both fc at once
                for i in range(bn):
                    nc.scalar.copy(out=slab[:, i, :, 0:SGW],
                                   in_=ps[i][:, :, 0:SGW])
                # stage 1: block maxes (DVE)
                for fc, (f0, fw, fwp) in enumerate(FCH):
                    sl = slab[0:fw, 0, 0, 0]
                    sl4 = bass.AP(tensor=sl.tensor,
                                  offset=sl.offset + PSW * fc,
                                  ap=[sl.ap[0], [2 * PSW, bn], [BLK, BPSG],
                                      [1, BLK]])
                    nc.vector.tensor_reduce(
                        out=ball[fc][0:fw, kk * BPSG:(kk + bn) * BPSG],
                        in_=sl4, axis=AX.X, op=ALU.max)
                kk += bn
                # group boundary: run stage 2 for completed groups
                gdone = kk // 8
                gprev = (kk - bn) // 8
                for g in range(gprev, min(gdone, ngrp)):
                    if 8 * (g + 1) <= kk or kk == nsg:
                        stage2_group(g)
                        # sentence chunks fully covered by finished groups
                        sdone = min(8 * (g + 1), nsg) * SPSG
                        for c in range(nchunk):
                            if (c + 1) * 128 <= sdone:
                                finish_chunk(c)
            if nsg % 8 != 0:
                stage2_group(nsg // 8)
            for c in range(nchunk):
                finish_chunk(c)

            # ---- bag mean + softmax ----
            bg_ps = t_psum.tile([128, NREL], f32, tag="tp", name="bg")
            for c in range(nchunk):
                cs = min(128, ns_pad - 128 * c)
                nc.tensor.matmul(
                    out=bg_ps[0:bags_cap, :],
                    lhsT=snorm_sb[c][0:cs, :],
                    rhs=lgs[c][0:cs, :],
                    start=(c == 0), stop=(c == nchunk - 1),
                    skip_group_check=True)

            t = singles.tile([128, NREL], f32, name="sm")
            nc.vector.tensor_copy(out=t[0:bags_cap, :],
                                  in_=bg_ps[0:bags_cap, :])
            nmax = singles.tile([128, 1], f32, name="nmax")
            nc.vector.reduce_max(out=nmax[0:bags_cap, :], in_=t[0:bags_cap, :],
                                 axis=AX.X, negate=True)
            ex = singles.tile([128, NREL], f32, name="ex")
            nc.scalar.activation(out=ex[0:bags_cap, :], in_=t[0:bags_cap, :],
                                 func=AF.Exp, bias=nmax[0:bags_cap, :],
                                 scale=1.0)
            ssum = singles.tile([128, 1], f32, name="ssum")
            nc.vector.reduce_sum(out=ssum[0:bags_cap, :],
                                 in_=ex[0:bags_cap, :], axis=AX.X)
            rcp = singles.tile([128, 1], f32, name="rcp")
            nc.vector.reciprocal(out=rcp[0:bags_cap, :],
                                 in_=ssum[0:bags_cap, :])
            res = singles.tile([128, NREL], f32, name="res")
            nc.vector.tensor_scalar_mul(res[0:bags_cap, :],
                                        ex[0:bags_cap, :],
                                        rcp[0:bags_cap, :])
            nc.sync.dma_start(out=out_d[:, :], in_=res[0:bags_cap, :])

    nc.compile()
    return nc


def _pad_edge(a):
    return np.concatenate([a[:, :1], a, a[:, -1:]], axis=1)


def kernel(**inputs):
    global LAST_RESULT
    sentences = np.asarray(inputs["sentences"]).astype(np.int64)
    pos1 = np.asarray(inputs["pos1"]).astype(np.int64)
    pos2 = np.asarray(inputs["pos2"]).astype(np.int64)
    masks = np.asarray(inputs["masks"]).astype(np.float32)
    bag_ids = np.asarray(inputs["bag_ids"]).astype(np.int64)
    word_emb = np.asarray(inputs["word_emb"]).astype(np.float32)
    pf1_emb = np.asarray(inputs["pf1_emb"]).astype(np.float32)
    pf2_emb = np.asarray(inputs["pf2_emb"]).astype(np.float32)
    conv_w = np.asarray(inputs["conv_w"]).astype(np.float32)
    conv_b = np.asarray(inputs["conv_b"]).astype(np.float32)
    dense_w = np.asarray(inputs["dense_w"]).astype(np.float32)
    dense_b = np.asarray(inputs["dense_b"]).astype(np.float32)

    # ---- balanced bag-boundary sharding ----
    counts = np.bincount(bag_ids, minlength=NBAGS)
    cum = np.concatenate([[0], np.cumsum(counts)])
    B = [0]
    for r in range(1, NCORES):
        B.append(int(np.argmin(np.abs(cum - N * r // NCORES))))
    B.append(NBAGS)
    for r in range(1, NCORES + 1):
        B[r] = max(B[r], B[r - 1])
    S = [int(cum[b]) for b in B]
    cnt = [S[r + 1] - S[r] for r in range(NCORES)]
    ncap = max(max(cnt), 1)
    nsg = (ncap + SPSG - 1) // SPSG
    ns_pad = SPSG * nsg
    ngrp = (nsg + HG - 1) // HG
    bags_cap = max(B[r + 1] - B[r] for r in range(NCORES))
    nchunk = (ns_pad + 127) // 128
    nblk = ns_pad * SBLK
    dcols = 2 * nblk
    nd = (dcols + 511) // 512
    TW = 2 * (HGW + 2)

    key = (nsg, ngrp, bags_cap, nchunk)
    if key not in _PROGRAM_CACHE:
        _PROGRAM_CACHE[key] = _build_program(nsg, ngrp, bags_cap, nchunk)
    nc = _PROGRAM_CACHE[key]

    # ---- shared parameter prep ----
    e8 = word_emb.astype(FP8)                    # [V, 300]
    e8main = np.ascontiguousarray(e8[:, :256])   # [V, 256]
    e8left = np.zeros((VOCAB, 44), FP8)
    e8left[:, :] = e8[:, 256:300]
    pf1_8 = pf1_emb.astype(FP8)                  # [240, 5]
    pf2_8 = pf2_emb.astype(FP8)

    wdr = np.zeros((3, 128, 2, 240), np.float32)
    for t in range(3):
        for i in range(2):
            wdr[t, :, i, :NF] = conv_w[:, i:256:2, t].T
    wdr = wdr.astype(FP8).reshape(3, 128, 480)

    wp = np.zeros((112, 2, 240), np.float32)
    for t in range(3):
        for i in range(2):
            wp[32 * t:32 * t + 22, i, :NF] = conv_w[:, 256 + i:300:2, t].T
            wp[96 + 5 * t:96 + 5 * t + 5, i, :NF] = conv_w[:, 300 + i:310:2, t].T
    wp[111, 0, :NF] = 1.0  # mask channel rides the center tap
    wp = wp.astype(FP8).reshape(112, 480)

    dwt = np.zeros((128, 6 * NREL), np.float32)
    for j in range(3):
        for fc, (f0, fw, fwp) in enumerate(FCH):
            dwt[:fw, (j * 2 + fc) * NREL:(j * 2 + fc + 1) * NREL] = \
                dense_w[:, j * NF + f0:j * NF + f0 + fw].T
    dwt = dwt.astype(F16)

    actb = np.zeros((128, 2), np.float32)
    for fc, (f0, fw, fwp) in enumerate(FCH):
        actb[:fw, fc] = conv_b[f0:f0 + fw]

    dbias = dense_b.reshape(1, NREL).astype(F16)
    fcounts = np.maximum(counts.astype(np.float32), 1.0)

    piece_all = masks.argmax(axis=1)                      # [N, 120]
    lo_all = (piece_all >= 1).argmax(axis=1)
    hi_all = (piece_all >= 2).argmax(axis=1)

    in_maps = []
    for r in range(NCORES):
        s0r, s1r = S[r], S[r + 1]
        nreal = s1r - s0r
        sent = np.zeros((ns_pad, L), np.int64)
        sent[:nreal] = sentences[s0r:s1r]
        p1 = np.zeros((ns_pad, L), np.int64)
        p1[:nreal] = pos1[s0r:s1r]
        p2 = np.zeros((ns_pad, L), np.int64)
        p2[:nreal] = pos2[s0r:s1r]

        sp = _pad_edge(sent)    # [ns_pad, 122]
        p1p = _pad_edge(p1)
        p2p = _pad_edge(p2)

        srcpos = np.zeros((ns_pad, SLOTS), np.int64)
        slotmask = np.full((ns_pad, SLOTS), -2 * MB, np.float32)
        bp = np.zeros((ns_pad, SBLK), np.int64)
        for i in range(nreal):
            a, b_, c_ = _sentence_layout(int(lo_all[s0r + i]),
                                         int(hi_all[s0r + i]))
            srcpos[i], slotmask[i], bp[i] = a, b_, c_

        tok = np.take_along_axis(sp, srcpos, axis=1)      # [ns_pad, 128]
        p1s = np.take_along_axis(p1p, srcpos, axis=1)
        p2s = np.take_along_axis(p2p, srcpos, axis=1)
        # cross-sentence fixup: last trailing slot carries the next
        # sentence's left-edge column
        tok[:-1, -1] = sp[1:, 0]
        p1s[:-1, -1] = p1p[1:, 0]
        p2s[:-1, -1] = p2p[1:, 0]

        Stot = ns_pad * SLOTS
        ghal_t = np.empty(Stot + 4, np.int64)
        ghal_t[2:-2] = tok.reshape(-1)
        ghal_t[:2] = sp[0, 0]
        ghal_t[-2:] = ghal_t[-3]
        ghal_1 = np.empty(Stot + 4, np.int64)
        ghal_1[2:-2] = p1s.reshape(-1)
        ghal_1[:2] = p1p[0, 0]
        ghal_1[-2:] = ghal_1[-3]
        ghal_2 = np.empty(Stot + 4, np.int64)
        ghal_2[2:-2] = p2s.reshape(-1)
        ghal_2[:2] = p2p[0, 0]
        ghal_2[-2:] = ghal_2[-3]
        ghal_m = np.full(Stot + 4, -2 * MB, np.float32)
        ghal_m[2:-2] = slotmask.reshape(-1)
        m8 = ghal_m.astype(FP8).view(np.uint8)

        gt8 = np.zeros((ngrp, 128, TW), np.uint8)
        pk8 = np.zeros((ngrp, 112, TW), np.uint8)
        for g in range(ngrp):
            u0 = HGW * g + 1            # ghal index of tile u=0 (slot -1)
            idx = np.arange(u0, u0 + HGW + 2)
            idx = np.minimum(idx, Stot + 3)
            arr = e8main[ghal_t[idx]].view(np.uint16)       # [4098, 128]
            gt8[g] = np.ascontiguousarray(arr.T).view(np.uint8).reshape(
                128, TW)
            for t in range(3):
                it = np.clip(idx + (t - 1), 0, Stot + 3)
                lv = e8left[ghal_t[it]].view(np.uint16)     # [4098, 22]
                pk8[g, 32 * t:32 * t + 22] = np.ascontiguousarray(
                    lv.T).view(np.uint8).reshape(22, TW)
                pfv = np.concatenate(
                    [pf1_8[ghal_1[it]], pf2_8[ghal_2[it]]],
                    axis=1).view(np.uint16)                  # [4098, 5]
                pk8[g, 96 + 5 * t:96 + 5 * t + 5] = np.ascontiguousarray(
                    pfv.T).view(np.uint8).reshape(5, TW)
            pk8[g, 111, 0::2] = m8[idx]
        gt8 = gt8.view(FP8)
        pk8 = pk8.view(FP8)

        # block deltas: d1 then d2, fp8 row
        drow = np.zeros((1, nd * 512), np.float32)
        d1 = np.where(bp == 1, MB, np.where(bp == 0, -MB, 0.0))
        d2 = np.where(bp == 2, MB, np.where(bp == 0, -MB, 0.0))
        drow[0, :nblk] = d1.reshape(-1)
        drow[0, nblk:2 * nblk] = d2.reshape(-1)
        drow = drow.astype(F16)

        snorm = np.zeros((nchunk * 128, bags_cap), np.float32)
        bags = bag_ids[s0r:s1r]
        snorm[np.arange(nreal), bags - B[r]] = 1.0 / fcounts[bags]
        snorm = snorm.astype(F16)

        in_maps.append({
            "gt8": gt8,
            "pk8": pk8,
            "wdr": wdr,
            "wp": wp,
            "drow": drow,
            "dwt": dwt,
            "actb": actb,
            "dbias": dbias,
            "snorm": snorm,
        })

    from concourse.bass_utils import run_bass_kernel_spmd

    trace = bool(int(os.environ.get("KERNEL_TRACE", "0")))
    res = run_bass_kernel_spmd(
        nc, in_maps, core_ids=list(range(NCORES)), trace=trace
    )
    LAST_RESULT = res

    out = np.zeros((NBAGS, NREL), np.float32)
    for r in range(NCORES):
        nb = B[r + 1] - B[r]
        if nb > 0:
            out[B[r]:B[r + 1]] = res.results[r]["out"][:nb].astype(np.float32)
    return out


if __name__ == "__main__":
    d = np.load("/root/problem/ref_inputs.npz")
    out = kernel(**{k: d[k] for k in d.files})
    print("out", out.shape, out.dtype)


# revision 4
# speedup vs baseline: 1.0363x; 1.0171x over previous
"""Trainium2 Bass kernel v3 for the PCNN bag-classification model.

Design:
  - Balanced bag-boundary sharding over 8 cores (no collectives).
  - Host ships the full fp8 DR-interleaved conv input stream (no on-device
    gather): channels 0..255 in the main stream tile, the 44 leftover word
    channels + 10 positional channels (pre-shifted per tap) + the mask
    channel in a packed tile.
  - Block-aligned piece layout: each sentence occupies 128 slots = 32 blocks
    of 4; the three PCNN pieces are padded to block boundaries.  Pad slots
    are killed by a conv mask channel (-8); real piece1/2 slots carry -4 so
    block maxes of foreign pieces always lose.
  - conv1d(k=3) as 4 DoubleRow fp8 matmuls per (subgroup=4 sentences,
    filter-chunk), weights batched across 3 subgroups to amortize LDWEIGHTS.
  - Hierarchical max-pool: scalar engine copies PSUM->SBUF f16 slabs, DVE
    reduces blocks of 4 (stage 1) into a per-core block-max array; per group
    of 8 subgroups, DVE computes the 3 phase maxima over block maxes with
    {-4,0,+4} block deltas (stage 2, 8x less data than slot level).
  - Block deltas are broadcast across partitions with a ones-matmul on the
    PE (instead of a 128x DMA broadcast).
  - Dense + bag-mean (segment mean as matmul with per-bag 1/count weights) +
    softmax on-chip, pipelined per 128-sentence chunk.
"""

import os
import sys

for _p in ("/opt/trn_rl_repo",):
    if _p not in sys.path:
        sys.path.insert(0, _p)

import numpy as np
import ml_dtypes

# ---------------- problem constants ----------------
N = 2048
L = 120
NCORES = 8
NF = 230
NREL = 53
NBAGS = 256
VOCAB = 100000
WD = 300
PD = 5

BLK = 8              # slots per block
SBLK = 17            # blocks per sentence
SLOTS = BLK * SBLK   # 136 slots per sentence
SPSG = 3             # sentences per subgroup (PSUM bank: 408 <= 512 f32)
SGW = SPSG * SLOTS   # 408 slots per subgroup
BPSG = SPSG * SBLK   # 51 blocks per subgroup
GRPW = 8 * SGW       # 3264 slots per group (8 subgroups)
HG = 4               # subgroups per fetch tile (half group)
HGW = HG * SGW       # 1632 slots per fetch tile
PSW = 512            # PSUM tile free width (per filter chunk)
MB = 4.0
FCH = [(0, 128, 128), (128, 102, 112)]  # (f0, fw_real, fw_pad)

FP8 = ml_dtypes.float8_e4m3
F16 = np.float16

_PROGRAM_CACHE = {}
_LAYOUT_CACHE = {}
LAST_RESULT = None


# ---------------- per-sentence block layout ----------------
def _sentence_layout(lo, hi):
    """Returns (srcpos[128] int64 into the 122-wide edge-padded arrays,
    slotmask[128] float {0,-MB,-2MB}, blockpiece[32] int64)."""
    key = (lo, hi)
    hit = _LAYOUT_CACHE.get(key)
    if hit is not None:
        return hit
    lens = [lo, hi - lo, L - hi]
    starts = [0, lo, hi]
    B0 = -(-lens[0] // BLK)
    B1 = -(-lens[1] // BLK)
    B2 = SBLK - B0 - B1
    assert B2 * BLK >= lens[2], (lo, hi)
    Bs = [B0, B1, B2]
    p = [Bs[i] * BLK - lens[i] for i in range(3)]
    sol = None
    for f0 in range(p[0] + 1):
        for f1 in range(p[1] + 1):
            for f2 in range(p[2] + 1):
                b0, b1, b2 = p[0] - f0, p[1] - f1, p[2] - f2
                if (b0 + f1) != 1 and (b1 + f2) != 1 and b2 >= 2:
                    sol = (f0, f1, f2)
                    break
            if sol:
                break
        if sol:
            break
    assert sol is not None, (lens, p)
    f = sol
    srcpos = np.zeros(SLOTS, np.int64)
    slotmask = np.full(SLOTS, -2 * MB, np.float32)
    blockpiece = np.zeros(SBLK, np.int64)
    s = 0
    bidx = 0
    rs, re = [], []
    for i in range(3):
        a, ln = starts[i], lens[i]
        blockpiece[bidx:bidx + Bs[i]] = i
        bidx += Bs[i]
        s += f[i]
        rs.append(s)
        srcpos[s:s + ln] = np.arange(a + 1, a + ln + 1)
        slotmask[s:s + ln] = 0.0 if i == 0 else -MB
        s += ln
        re.append(s)
        s += p[i] - f[i]
    assert s == SLOTS
    srcpos[0:rs[0]] = 0
    for i in range(2):
        r0, r1 = re[i], rs[i + 1]
        if r1 > r0:
            srcpos[r0:r1] = starts[i] + lens[i] + 1
            srcpos[r1 - 1] = starts[i + 1]
    srcpos[re[2]:SLOTS] = L + 1
    out = (srcpos, slotmask, blockpiece)
    _LAYOUT_CACHE[key] = out
    return out


# ---------------- device program ----------------
def _build_program(nsg, ngrp, bags_cap, nchunk):
    import concourse.bass as bass
    import concourse.mybir as mybir
    import concourse.tile as tile
    from concourse import bacc

    f32 = mybir.dt.float32
    f16 = mybir.dt.float16
    fp8 = mybir.dt.float8e4
    AF = mybir.ActivationFunctionType
    AX = mybir.AxisListType
    ALU = mybir.AluOpType
    DR = mybir.MatmulPerfMode.DoubleRow

    ns_pad = SPSG * nsg
    nblk = ns_pad * SBLK            # total blocks per core
    dcols = 2 * nblk                # delta row columns (2 phases)
    nd = (dcols + 511) // 512       # delta broadcast chunks
    TW = 2 * (HGW + 2)              # stream tile bytes per partition
    GP_ON = bool(int(os.environ.get("KERNEL_GP", "1")))

    nc = bacc.Bacc(
        "TRN2", target_bir_lowering=False, debug=False, num_devices=NCORES,
        num_swdge_queues=1,
    )

    gt_d = nc.dram_tensor("gt8", [ngrp, 128, TW], fp8, kind="ExternalInput").ap()
    pk_d = nc.dram_tensor("pk8", [ngrp, 112, TW], fp8, kind="ExternalInput").ap()
    wdr_d = nc.dram_tensor("wdr", [3, 128, 2 * 240], fp8,
                           kind="ExternalInput").ap()
    wp_d = nc.dram_tensor("wp", [112, 2 * 240], fp8, kind="ExternalInput").ap()
    drow_d = nc.dram_tensor("drow", [1, nd * 512], f16,
                            kind="ExternalInput").ap()
    dwt_d = nc.dram_tensor("dwt", [128, 6 * NREL], f16,
                           kind="ExternalInput").ap()
    actb_d = nc.dram_tensor("actb", [128, 2], f32, kind="ExternalInput").ap()
    dbias_d = nc.dram_tensor("dbias", [1, NREL], f16, kind="ExternalInput").ap()
    snorm_d = nc.dram_tensor("snorm", [nchunk * 128, bags_cap], f16,
                             kind="ExternalInput").ap()
    out_d = nc.dram_tensor("out", [bags_cap, NREL], f32,
                           kind="ExternalOutput").ap()

    with tile.TileContext(nc) as tc:
        import contextlib

        ctx = contextlib.ExitStack()
        with ctx:
            singles = ctx.enter_context(tc.tile_pool(name="singles", bufs=1))

            wdr_sb = [singles.tile([128, 2, 240], fp8, name=f"wdr{t}")
                      for t in range(3)]
            wp_sb = singles.tile([112, 2, 240], fp8)
            dwt_sb = singles.tile([128, 6 * NREL], f16)
            actb_sb = singles.tile([128, 2], f32)
            dbias_sb = singles.tile([1, NREL], f16)
            snorm_sb = [singles.tile([128, bags_cap], f16, name=f"sn{c}")
                        for c in range(nchunk)]
            ones_sb = singles.tile([1, 128], f16)
            ball = [singles.tile([128, nblk], f16, name=f"ball{c}")
                    for c in range(2)]
            tmp1 = singles.tile([128, 3 * 204], f16, name="tmp1")
            tmp2 = singles.tile([128, 3 * 102], f16, name="tmp2")
            delta_sb = singles.tile([128, 2, nblk], f16)
            scr = [singles.tile([128, 8 * BPSG], f16, name=f"scr{c}")
                   for c in range(2)]
            gtmp = singles.tile([128, 3, 256], f32, name="gtmp")
            pooled = [singles.tile([128, 3, ns_pad], f16, name=f"pool{c}")
                      for c in range(2)]
            pr = [singles.tile([128, 3, ns_pad], f16, name=f"pr{c}")
                  for c in range(2)]
            lgs = [singles.tile([128, NREL], f16, name=f"lgs{c}")
                   for c in range(nchunk)]

            gt_pool = ctx.enter_context(tc.tile_pool(name="gt", bufs=4))
            pk_pool = ctx.enter_context(tc.tile_pool(name="pk", bufs=4))
            cp_pool = ctx.enter_context(tc.tile_pool(name="cp", bufs=4))
            cv_psum = ctx.enter_context(
                tc.tile_pool(name="cv", bufs=3, space="PSUM"))
            t_psum = ctx.enter_context(
                tc.tile_pool(name="tp", bufs=2, space="PSUM"))

            gt_tiles = {}
            pk_tiles = {}

            def fetch_group(g):
                if g in gt_tiles or g >= ngrp:
                    return
                gt = gt_pool.tile([128, TW], fp8, tag="gt", name=f"gt{g}")
                nc.sync.dma_start(out=gt[:, :], in_=gt_d[g, :, :])
                pk = pk_pool.tile([112, TW], fp8, tag="pk", name=f"pk{g}")
                nc.sync.dma_start(out=pk[:, :], in_=pk_d[g, :, :])
                gt_tiles[g] = gt
                pk_tiles[g] = pk

            # conv inputs first so the PE can start ASAP
            for t in range(3):
                nc.sync.dma_start(out=wdr_sb[t][:, :, :], in_=wdr_d[t, :, :])
            nc.sync.dma_start(out=wp_sb[:, :, :], in_=wp_d[:, :])
            fetch_group(0)
            fetch_group(1)
            fetch_group(2)
            nc.sync.dma_start(out=dwt_sb[:, :], in_=dwt_d[:, :])
            nc.sync.dma_start(out=actb_sb[:, :], in_=actb_d[:, :])
            nc.sync.dma_start(out=dbias_sb[:, :], in_=dbias_d[:, :])
            for c in range(nchunk):
                nc.sync.dma_start(out=snorm_sb[c][:, :],
                                  in_=snorm_d[c * 128:(c + 1) * 128, :])
            nc.vector.memset(ones_sb[:, :], 1.0)

            # ---- broadcast the block deltas across partitions via DMA ----
            dview = delta_sb[:, 0, 0]
            dflat = bass.AP(tensor=dview.tensor, offset=dview.offset,
                            ap=[dview.ap[0], [1, 2 * nblk]])
            nc.sync.dma_start(
                out=dflat,
                in_=drow_d[:, 0:2 * nblk].to_broadcast((128, 2 * nblk)))

            BATCH = 3
            lgs_done = [False] * nchunk

            def stage2_group(g):
                """Phase maxima for group g's 32 sentences (DVE)."""
                sg0 = 8 * g
                sgn = min(8, nsg - sg0)
                b0 = sg0 * BPSG         # first block index
                nb = sgn * BPSG         # blocks in this group
                s0 = SPSG * sg0         # first sentence
                nsent = SPSG * sgn
                for fc in range(2):
                    fw = FCH[fc][1]
                    bsl = ball[fc][0:fw, b0:b0 + nb]
                    bs3 = bass.AP(tensor=bsl.tensor, offset=bsl.offset,
                                  ap=[bsl.ap[0], [SBLK, nsent], [1, SBLK]])
                    nc.vector.tensor_reduce(
                        out=pooled[fc][0:fw, 0, s0:s0 + nsent],
                        in_=bs3, axis=AX.X, op=ALU.max)
                    teng = nc.gpsimd if (GP_ON and fc == 1) else nc.vector
                    for ph in range(2):
                        sc = scr[fc][0:fw, 0:nb]
                        teng.tensor_tensor(
                            out=sc, in0=bsl,
                            in1=delta_sb[0:fw, ph, b0:b0 + nb],
                            op=ALU.add)
                        sc3 = bass.AP(tensor=sc.tensor, offset=sc.offset,
                                      ap=[sc.ap[0], [SBLK, nsent], [1, SBLK]])
                        nc.vector.tensor_reduce(
                            out=pooled[fc][0:fw, 1 + ph, s0:s0 + nsent],
                            in_=sc3, axis=AX.X, op=ALU.max)

            def finish_chunk(c):
                """relu + logits matmul for sentence chunk c (128 sentences)."""
                if lgs_done[c]:
                    return
                lgs_done[c] = True
                cs = min(128, ns_pad - 128 * c)
                for fc in range(2):
                    fw = FCH[fc][1]
                    nc.scalar.activation(
                        out=pr[fc][0:fw, :, 128 * c:128 * c + cs],
                        in_=pooled[fc][0:fw, :, 128 * c:128 * c + cs],
                        func=AF.Relu, bias=actb_sb[0:fw, fc:fc + 1], scale=1.0)
                lg_ps = t_psum.tile([128, NREL], f32, tag="tp", name=f"lg{c}")
                nmm = 0
                for j in range(3):
                    for fc, (f0, fw, fwp) in enumerate(FCH):
                        nc.tensor.matmul(
                            out=lg_ps[0:cs, :],
                            lhsT=pr[fc][0:fw, j, 128 * c:128 * c + cs],
                            rhs=dwt_sb[0:fw, (j * 2 + fc) * NREL:
                                       (j * 2 + fc + 1) * NREL],
                            start=(nmm == 0), stop=False,
                            skip_group_check=True)
                        nmm += 1
                nc.tensor.matmul(
                    out=lg_ps[0:cs, :],
                    lhsT=ones_sb[0:1, 0:cs],
                    rhs=dbias_sb[0:1, :],
                    start=False, stop=True, skip_group_check=True)
                nc.scalar.copy(out=lgs[c][0:cs, :], in_=lg_ps[0:cs, :])

            kk = 0
            while kk < nsg:
                bn = min(BATCH, nsg - kk)
                # prefetch a future fetch tile
                fetch_group(kk // HG + 3)
                ps = []
                slab = cp_pool.tile([128, bn, 2, PSW], f16, tag="cp",
                                    name=f"sl{kk}")
                for i in range(bn):
                    ps.append(cv_psum.tile([128, 2, PSW], f32, tag="cv",
                                           name=f"cv{kk + i}"))
                # conv matmuls, weights-major for LDW amortization
                for fc, (f0, fw, fwp) in enumerate(FCH):
                    for t in range(4):
                        for i in range(bn):
                            g = (kk + i) // HG
                            l = (kk + i) % HG
                            gtt = gt_tiles[g]
                            if t < 3:
                                gb = gtt[:, 0]
                                rhs = bass.AP(
                                    tensor=gb.tensor,
                                    offset=gb.offset + 2 * (SGW * l + t),
                                    ap=[gb.ap[0], [1, 2], [2, SGW]])
                                nc.tensor.matmul(
                                    out=ps[i][0:fwp, fc, 0:SGW],
                                    lhsT=wdr_sb[t][:, :, f0:f0 + fwp],
                                    rhs=rhs, start=(t == 0), stop=False,
                                    perf_mode=DR, skip_group_check=True)
                            else:
                                pkt = pk_tiles[g]
                                pb = pkt[0:112, 0]
                                rhsp = bass.AP(
                                    tensor=pb.tensor,
                                    offset=pb.offset + 2 * (SGW * l + 1),
                                    ap=[[pb.ap[0][0], 112], [1, 2], [2, SGW]])
                                nc.tensor.matmul(
                                    out=ps[i][0:fwp, fc, 0:SGW],
                                    lhsT=wp_sb[0:112, :, f0:f0 + fwp],
                                    rhs=rhsp, start=False, stop=True,
                                    perf_mode=DR, skip_group_check=True)
                # PSUM -> SBUF f16 slab (scalar engine), both fc at once
                for i in range(bn):
                    nc.scalar.copy(out=slab[:, i, :, 0:SGW],
                                   in_=ps[i][:, :, 0:SGW])
                # stage 1: block maxes (DVE) as a 3-level pairwise-max tree;
                # block-half pairing keeps innermost runs contiguous so the
                # first two levels hit the 2x_1p DVE mode.
                for fc, (f0, fw, fwp) in enumerate(FCH):
                    sl = slab[0:fw, 0, 0, 0]
                    pp = sl.ap[0]
                    o = sl.offset + PSW * fc
                    t1 = tmp1[0:fw, 0]
                    t2 = tmp2[0:fw, 0]
                    p1 = t1.ap[0]
                    p2 = t2.ap[0]
                    nc.vector.tensor_tensor(
                        out=bass.AP(tensor=t1.tensor, offset=t1.offset,
                                    ap=[p1, [204, bn], [4, BPSG], [1, 4]]),
                        in0=bass.AP(tensor=sl.tensor, offset=o,
                                    ap=[pp, [2 * PSW, bn], [8, BPSG], [1, 4]]),
                        in1=bass.AP(tensor=sl.tensor, offset=o + 4,
                                    ap=[pp, [2 * PSW, bn], [8, BPSG], [1, 4]]),
                        op=ALU.max)
                    nc.vector.tensor_tensor(
                        out=bass.AP(tensor=t2.tensor, offset=t2.offset,
                                    ap=[p2, [102, bn], [2, BPSG], [1, 2]]),
                        in0=bass.AP(tensor=t1.tensor, offset=t1.offset,
                                    ap=[p1, [204, bn], [4, BPSG], [1, 2]]),
                        in1=bass.AP(tensor=t1.tensor, offset=t1.offset + 2,
                                    ap=[p1, [204, bn], [4, BPSG], [1, 2]]),
                        op=ALU.max)
                    bo = ball[fc][0:fw, kk * BPSG]
                    nc.vector.tensor_tensor(
                        out=bass.AP(tensor=bo.tensor, offset=bo.offset,
                                    ap=[bo.ap[0], [BPSG, bn], [1, BPSG]]),
                        in0=bass.AP(tensor=t2.tensor, offset=t2.offset,
                                    ap=[p2, [102, bn], [2, BPSG]]),
                        in1=bass.AP(tensor=t2.tensor, offset=t2.offset + 1,
                                    ap=[p2, [102, bn], [2, BPSG]]),
                        op=ALU.max)
                kk += bn
                # group boundary: run stage 2 for completed groups
                gdone = kk // 8
                gprev = (kk - bn) // 8
                for g in range(gprev, min(gdone, ngrp)):
                    if 8 * (g + 1) <= kk or kk == nsg:
                        stage2_group(g)
                        # sentence chunks fully covered by finished groups
                        sdone = min(8 * (g + 1), nsg) * SPSG
                        for c in range(nchunk):
                            if (c + 1) * 128 <= sdone:
                                finish_chunk(c)
            if nsg % 8 != 0:
                stage2_group(nsg // 8)
            for c in range(nchunk):
                finish_chunk(c)

            # ---- bag mean + softmax ----
            bg_ps = t_psum.tile([128, NREL], f32, tag="tp", name="bg")
            for c in range(nchunk):
                cs = min(128, ns_pad - 128 * c)
                nc.tensor.matmul(
                    out=bg_ps[0:bags_cap, :],
                    lhsT=snorm_sb[c][0:cs, :],
                    rhs=lgs[c][0:cs, :],
                    start=(c == 0), stop=(c == nchunk - 1),
                    skip_group_check=True)

            t = singles.tile([128, NREL], f32, name="sm")
            nc.vector.tensor_copy(out=t[0:bags_cap, :],
                                  in_=bg_ps[0:bags_cap, :])
            nmax = singles.tile([128, 1], f32, name="nmax")
            nc.vector.reduce_max(out=nmax[0:bags_cap, :], in_=t[0:bags_cap, :],
                                 axis=AX.X, negate=True)
            ex = singles.tile([128, NREL], f32, name="ex")
            nc.scalar.activation(out=ex[0:bags_cap, :], in_=t[0:bags_cap, :],
                                 func=AF.Exp, bias=nmax[0:bags_cap, :],
                                 scale=1.0)
            ssum = singles.tile([128, 1], f32, name="ssum")
            nc.vector.reduce_sum(out=ssum[0:bags_cap, :],
                                 in_=ex[0:bags_cap, :], axis=AX.X)
            rcp = singles.tile([128, 1], f32, name="rcp")
            nc.vector.reciprocal(out=rcp[0:bags_cap, :],
                                 in_=ssum[0:bags_cap, :])
            res = singles.tile([128, NREL], f32, name="res")
            nc.vector.tensor_scalar_mul(res[0:bags_cap, :],
                                        ex[0:bags_cap, :],
                                        rcp[0:bags_cap, :])
            nc.sync.dma_start(out=out_d[:, :], in_=res[0:bags_cap, :])

    nc.compile()
    return nc


def _pad_edge(a):
    return np.concatenate([a[:, :1], a, a[:, -1:]], axis=1)


def kernel(**inputs):
    global LAST_RESULT
    sentences = np.asarray(inputs["sentences"]).astype(np.int64)
    pos1 = np.asarray(inputs["pos1"]).astype(np.int64)
    pos2 = np.asarray(inputs["pos2"]).astype(np.int64)
    masks = np.asarray(inputs["masks"]).astype(np.float32)
    bag_ids = np.asarray(inputs["bag_ids"]).astype(np.int64)
    word_emb = np.asarray(inputs["word_emb"]).astype(np.float32)
    pf1_emb = np.asarray(inputs["pf1_emb"]).astype(np.float32)
    pf2_emb = np.asarray(inputs["pf2_emb"]).astype(np.float32)
    conv_w = np.asarray(inputs["conv_w"]).astype(np.float32)
    conv_b = np.asarray(inputs["conv_b"]).astype(np.float32)
    dense_w = np.asarray(inputs["dense_w"]).astype(np.float32)
    dense_b = np.asarray(inputs["dense_b"]).astype(np.float32)

    # ---- balanced bag-boundary sharding ----
    counts = np.bincount(bag_ids, minlength=NBAGS)
    cum = np.concatenate([[0], np.cumsum(counts)])
    B = [0]
    for r in range(1, NCORES):
        B.append(int(np.argmin(np.abs(cum - N * r // NCORES))))
    B.append(NBAGS)
    for r in range(1, NCORES + 1):
        B[r] = max(B[r], B[r - 1])
    S = [int(cum[b]) for b in B]
    cnt = [S[r + 1] - S[r] for r in range(NCORES)]
    ncap = max(max(cnt), 1)
    nsg = (ncap + SPSG - 1) // SPSG
    ns_pad = SPSG * nsg
    ngrp = (nsg + HG - 1) // HG
    bags_cap = max(B[r + 1] - B[r] for r in range(NCORES))
    nchunk = (ns_pad + 127) // 128
    nblk = ns_pad * SBLK
    dcols = 2 * nblk
    nd = (dcols + 511) // 512
    TW = 2 * (HGW + 2)

    key = (nsg, ngrp, bags_cap, nchunk)
    if key not in _PROGRAM_CACHE:
        _PROGRAM_CACHE[key] = _build_program(nsg, ngrp, bags_cap, nchunk)
    nc = _PROGRAM_CACHE[key]

    # ---- shared parameter prep ----
    e8 = word_emb.astype(FP8)                    # [V, 300]
    e8main = np.ascontiguousarray(e8[:, :256])   # [V, 256]
    e8left = np.zeros((VOCAB, 44), FP8)
    e8left[:, :] = e8[:, 256:300]
    pf1_8 = pf1_emb.astype(FP8)                  # [240, 5]
    pf2_8 = pf2_emb.astype(FP8)

    wdr = np.zeros((3, 128, 2, 240), np.float32)
    for t in range(3):
        for i in range(2):
            wdr[t, :, i, :NF] = conv_w[:, i:256:2, t].T
    wdr = wdr.astype(FP8).reshape(3, 128, 480)

    wp = np.zeros((112, 2, 240), np.float32)
    for t in range(3):
        for i in range(2):
            wp[32 * t:32 * t + 22, i, :NF] = conv_w[:, 256 + i:300:2, t].T
            wp[96 + 5 * t:96 + 5 * t + 5, i, :NF] = conv_w[:, 300 + i:310:2, t].T
    wp[111, 0, :NF] = 1.0  # mask channel rides the center tap
    wp = wp.astype(FP8).reshape(112, 480)

    dwt = np.zeros((128, 6 * NREL), np.float32)
    for j in range(3):
        for fc, (f0, fw, fwp) in enumerate(FCH):
            dwt[:fw, (j * 2 + fc) * NREL:(j * 2 + fc + 1) * NREL] = \
                dense_w[:, j * NF + f0:j * NF + f0 + fw].T
    dwt = dwt.astype(F16)

    actb = np.zeros((128, 2), np.float32)
    for fc, (f0, fw, fwp) in enumerate(FCH):
        actb[:fw, fc] = conv_b[f0:f0 + fw]

    dbias = dense_b.reshape(1, NREL).astype(F16)
    fcounts = np.maximum(counts.astype(np.float32), 1.0)

    piece_all = masks.argmax(axis=1)                      # [N, 120]
    lo_all = (piece_all >= 1).argmax(axis=1)
    hi_all = (piece_all >= 2).argmax(axis=1)

    in_maps = []
    for r in range(NCORES):
        s0r, s1r = S[r], S[r + 1]
        nreal = s1r - s0r
        sent = np.zeros((ns_pad, L), np.int64)
        sent[:nreal] = sentences[s0r:s1r]
        p1 = np.zeros((ns_pad, L), np.int64)
        p1[:nreal] = pos1[s0r:s1r]
        p2 = np.zeros((ns_pad, L), np.int64)
        p2[:nreal] = pos2[s0r:s1r]

        sp = _pad_edge(sent)    # [ns_pad, 122]
        p1p = _pad_edge(p1)
        p2p = _pad_edge(p2)

        srcpos = np.zeros((ns_pad, SLOTS), np.int64)
        slotmask = np.full((ns_pad, SLOTS), -2 * MB, np.float32)
        bp = np.zeros((ns_pad, SBLK), np.int64)
        for i in range(nreal):
            a, b_, c_ = _sentence_layout(int(lo_all[s0r + i]),
                                         int(hi_all[s0r + i]))
            srcpos[i], slotmask[i], bp[i] = a, b_, c_

        tok = np.take_along_axis(sp, srcpos, axis=1)      # [ns_pad, 128]
        p1s = np.take_along_axis(p1p, srcpos, axis=1)
        p2s = np.take_along_axis(p2p, srcpos, axis=1)
        # cross-sentence fixup: last trailing slot carries the next
        # sentence's left-edge column
        tok[:-1, -1] = sp[1:, 0]
        p1s[:-1, -1] = p1p[1:, 0]
        p2s[:-1, -1] = p2p[1:, 0]

        Stot = ns_pad * SLOTS
        ghal_t = np.empty(Stot + 4, np.int64)
        ghal_t[2:-2] = tok.reshape(-1)
        ghal_t[:2] = sp[0, 0]
        ghal_t[-2:] = ghal_t[-3]
        ghal_1 = np.empty(Stot + 4, np.int64)
        ghal_1[2:-2] = p1s.reshape(-1)
        ghal_1[:2] = p1p[0, 0]
        ghal_1[-2:] = ghal_1[-3]
        ghal_2 = np.empty(Stot + 4, np.int64)
        ghal_2[2:-2] = p2s.reshape(-1)
        ghal_2[:2] = p2p[0, 0]
        ghal_2[-2:] = ghal_2[-3]
        ghal_m = np.full(Stot + 4, -2 * MB, np.float32)
        ghal_m[2:-2] = slotmask.reshape(-1)
        m8 = ghal_m.astype(FP8).view(np.uint8)

        gt8 = np.zeros((ngrp, 128, TW), np.uint8)
        pk8 = np.zeros((ngrp, 112, TW), np.uint8)
        for g in range(ngrp):
            u0 = HGW * g + 1            # ghal index of tile u=0 (slot -1)
            idx = np.arange(u0, u0 + HGW + 2)
            idx = np.minimum(idx, Stot + 3)
            arr = e8main[ghal_t[idx]].view(np.uint16)       # [4098, 128]
            gt8[g] = np.ascontiguousarray(arr.T).view(np.uint8).reshape(
                128, TW)
            for t in range(3):
                it = np.clip(idx + (t - 1), 0, Stot + 3)
                lv = e8left[ghal_t[it]].view(np.uint16)     # [4098, 22]
                pk8[g, 32 * t:32 * t + 22] = np.ascontiguousarray(
                    lv.T).view(np.uint8).reshape(22, TW)
                pfv = np.concatenate(
                    [pf1_8[ghal_1[it]], pf2_8[ghal_2[it]]],
                    axis=1).view(np.uint16)                  # [4098, 5]
                pk8[g, 96 + 5 * t:96 + 5 * t + 5] = np.ascontiguousarray(
                    pfv.T).view(np.uint8).reshape(5, TW)
            pk8[g, 111, 0::2] = m8[idx]
        gt8 = gt8.view(FP8)
        pk8 = pk8.view(FP8)

        # block deltas: d1 then d2, fp8 row
        drow = np.zeros((1, nd * 512), np.float32)
        d1 = np.where(bp == 1, MB, np.where(bp == 0, -MB, 0.0))
        d2 = np.where(bp == 2, MB, np.where(bp == 0, -MB, 0.0))
        drow[0, :nblk] = d1.reshape(-1)
        drow[0, nblk:2 * nblk] = d2.reshape(-1)
        drow = drow.astype(F16)

        snorm = np.zeros((nchunk * 128, bags_cap), np.float32)
        bags = bag_ids[s0r:s1r]
        snorm[np.arange(nreal), bags - B[r]] = 1.0 / fcounts[bags]
        snorm = snorm.astype(F16)

        in_maps.append({
            "gt8": gt8,
            "pk8": pk8,
            "wdr": wdr,
            "wp": wp,
            "drow": drow,
            "dwt": dwt,
            "actb": actb,
            "dbias": dbias,
            "snorm": snorm,
        })

    from concourse.bass_utils import run_bass_kernel_spmd

    trace = bool(int(os.environ.get("KERNEL_TRACE", "0")))
    res = run_bass_kernel_spmd(
        nc, in_maps, core_ids=list(range(NCORES)), trace=trace
    )
    LAST_RESULT = res

    out = np.zeros((NBAGS, NREL), np.float32)
    for r in range(NCORES):
        nb = B[r + 1] - B[r]
        if nb > 0:
            out[B[r]:B[r + 1]] = res.results[r]["out"][:nb].astype(np.float32)
    return out


if __name__ == "__main__":
    d = np.load("/root/problem/ref_inputs.npz")
    out = kernel(**{k: d[k] for k in d.files})
    print("out", out.shape, out.dtype)


# revision 6
# speedup vs baseline: 1.0553x; 1.0183x over previous
"""Trainium2 Bass kernel v3 for the PCNN bag-classification model.

Design:
  - Balanced bag-boundary sharding over 8 cores (no collectives).
  - Host ships the full fp8 DR-interleaved conv input stream (no on-device
    gather): channels 0..255 in the main stream tile, the 44 leftover word
    channels + 10 positional channels (pre-shifted per tap) + the mask
    channel in a packed tile.
  - Block-aligned piece layout: each sentence occupies 128 slots = 32 blocks
    of 4; the three PCNN pieces are padded to block boundaries.  Pad slots
    are killed by a conv mask channel (-8); real piece1/2 slots carry -4 so
    block maxes of foreign pieces always lose.
  - conv1d(k=3) as 4 DoubleRow fp8 matmuls per (subgroup=4 sentences,
    filter-chunk), weights batched across 3 subgroups to amortize LDWEIGHTS.
  - Hierarchical max-pool: scalar engine copies PSUM->SBUF f16 slabs, DVE
    reduces blocks of 4 (stage 1) into a per-core block-max array; per group
    of 8 subgroups, DVE computes the 3 phase maxima over block maxes with
    {-4,0,+4} block deltas (stage 2, 8x less data than slot level).
  - Block deltas are broadcast across partitions with a ones-matmul on the
    PE (instead of a 128x DMA broadcast).
  - Dense + bag-mean (segment mean as matmul with per-bag 1/count weights) +
    softmax on-chip, pipelined per 128-sentence chunk.
"""

import os
import sys

for _p in ("/opt/trn_rl_repo",):
    if _p not in sys.path:
        sys.path.insert(0, _p)

import numpy as np
import ml_dtypes

# ---------------- problem constants ----------------
N = 2048
L = 120
NCORES = 8
NF = 230
NREL = 53
NBAGS = 256
VOCAB = 100000
WD = 300
PD = 5

BLK = 8              # slots per block
SBLK = 17            # blocks per sentence
SLOTS = BLK * SBLK   # 136 slots per sentence
SPSG = 3             # sentences per subgroup (PSUM bank: 408 <= 512 f32)
SGW = SPSG * SLOTS   # 408 slots per subgroup
BPSG = SPSG * SBLK   # 51 blocks per subgroup
GRPW = 8 * SGW       # 3264 slots per group (8 subgroups)
HG = 2               # subgroups per fetch tile
HGW = HG * SGW       # 816 slots per fetch tile
PSW = 512            # PSUM tile free width (per filter chunk)
MB = 4.0
FCH = [(0, 128, 128), (128, 102, 112)]  # (f0, fw_real, fw_pad)

FP8 = ml_dtypes.float8_e4m3
F16 = np.float16

_PROGRAM_CACHE = {}
_LAYOUT_CACHE = {}
LAST_RESULT = None


# ---------------- per-sentence block layout ----------------
def _sentence_layout(lo, hi):
    """Returns (srcpos[128] int64 into the 122-wide edge-padded arrays,
    slotmask[128] float {0,-MB,-2MB}, blockpiece[32] int64)."""
    key = (lo, hi)
    hit = _LAYOUT_CACHE.get(key)
    if hit is not None:
        return hit
    lens = [lo, hi - lo, L - hi]
    starts = [0, lo, hi]
    B0 = -(-lens[0] // BLK)
    B1 = -(-lens[1] // BLK)
    B2 = SBLK - B0 - B1
    assert B2 * BLK >= lens[2], (lo, hi)
    Bs = [B0, B1, B2]
    p = [Bs[i] * BLK - lens[i] for i in range(3)]
    sol = None
    for f0 in range(p[0] + 1):
        for f1 in range(p[1] + 1):
            for f2 in range(p[2] + 1):
                b0, b1, b2 = p[0] - f0, p[1] - f1, p[2] - f2
                if (b0 + f1) != 1 and (b1 + f2) != 1 and b2 >= 2:
                    sol = (f0, f1, f2)
                    break
            if sol:
                break
        if sol:
            break
    assert sol is not None, (lens, p)
    f = sol
    srcpos = np.zeros(SLOTS, np.int64)
    slotmask = np.full(SLOTS, -2 * MB, np.float32)
    blockpiece = np.zeros(SBLK, np.int64)
    s = 0
    bidx = 0
    rs, re = [], []
    for i in range(3):
        a, ln = starts[i], lens[i]
        blockpiece[bidx:bidx + Bs[i]] = i
        bidx += Bs[i]
        s += f[i]
        rs.append(s)
        srcpos[s:s + ln] = np.arange(a + 1, a + ln + 1)
        slotmask[s:s + ln] = 0.0 if i == 0 else -MB
        s += ln
        re.append(s)
        s += p[i] - f[i]
    assert s == SLOTS
    srcpos[0:rs[0]] = 0
    for i in range(2):
        r0, r1 = re[i], rs[i + 1]
        if r1 > r0:
            srcpos[r0:r1] = starts[i] + lens[i] + 1
            srcpos[r1 - 1] = starts[i + 1]
    srcpos[re[2]:SLOTS] = L + 1
    out = (srcpos, slotmask, blockpiece)
    _LAYOUT_CACHE[key] = out
    return out


# ---------------- device program ----------------
def _build_program(nsg, ngrp, bags_cap, nchunk):
    import concourse.bass as bass
    import concourse.mybir as mybir
    import concourse.tile as tile
    from concourse import bacc

    f32 = mybir.dt.float32
    f16 = mybir.dt.float16
    fp8 = mybir.dt.float8e4
    AF = mybir.ActivationFunctionType
    AX = mybir.AxisListType
    ALU = mybir.AluOpType
    DR = mybir.MatmulPerfMode.DoubleRow

    ns_pad = SPSG * nsg
    nblk = ns_pad * SBLK            # total blocks per core
    dcols = 2 * nblk                # delta row columns (2 phases)
    nd = (dcols + 511) // 512       # delta broadcast chunks
    TW = 2 * (HGW + 2)              # stream tile bytes per partition
    GP_ON = bool(int(os.environ.get("KERNEL_GP", "1")))

    nc = bacc.Bacc(
        "TRN2", target_bir_lowering=False, debug=False, num_devices=NCORES,
        num_swdge_queues=1,
    )

    gt_d = nc.dram_tensor("gt8", [ngrp, 128, TW], fp8, kind="ExternalInput").ap()
    pk_d = nc.dram_tensor("pk8", [ngrp, 112, TW], fp8, kind="ExternalInput").ap()
    wall_d = nc.dram_tensor("wall", [128, 4 * 480], fp8,
                            kind="ExternalInput").ap()
    drow_d = nc.dram_tensor("drow", [1, nd * 512], f16,
                            kind="ExternalInput").ap()
    dwt_d = nc.dram_tensor("dwt", [128, 6 * NREL], f16,
                           kind="ExternalInput").ap()
    actb_d = nc.dram_tensor("actb", [128, 2], f32, kind="ExternalInput").ap()
    dbias_d = nc.dram_tensor("dbias", [1, NREL], f16, kind="ExternalInput").ap()
    snorm_d = nc.dram_tensor("snorm", [nchunk * 128, bags_cap], f16,
                             kind="ExternalInput").ap()
    out_d = nc.dram_tensor("out", [bags_cap, NREL], f32,
                           kind="ExternalOutput").ap()

    with tile.TileContext(nc) as tc:
        import contextlib

        ctx = contextlib.ExitStack()
        with ctx:
            singles = ctx.enter_context(tc.tile_pool(name="singles", bufs=1))

            wall_sb = singles.tile([128, 4, 2, 240], fp8, name="wall")
            dwt_sb = singles.tile([128, 6 * NREL], f16)
            actb_sb = singles.tile([128, 2], f32)
            dbias_sb = singles.tile([1, NREL], f16)
            snorm_sb = [singles.tile([128, bags_cap], f16, name=f"sn{c}")
                        for c in range(nchunk)]
            ones_sb = singles.tile([1, 128], f16)
            ball = [singles.tile([128, nblk], f16, name=f"ball{c}")
                    for c in range(2)]
            tmp1 = singles.tile([128, 3 * 204], f16, name="tmp1")
            tmp2 = singles.tile([128, 3 * 102], f16, name="tmp2")
            delta_sb = singles.tile([128, 2, nblk], f16)
            scr = [singles.tile([128, 8 * BPSG], f16, name=f"scr{c}")
                   for c in range(2)]
            gtmp = singles.tile([128, 3, 256], f32, name="gtmp")
            pooled = [singles.tile([128, 3, ns_pad], f16, name=f"pool{c}")
                      for c in range(2)]
            pr = [singles.tile([128, 3, ns_pad], f16, name=f"pr{c}")
                  for c in range(2)]
            lgs = [singles.tile([128, NREL], f16, name=f"lgs{c}")
                   for c in range(nchunk)]

            gt_pool = ctx.enter_context(tc.tile_pool(name="gt", bufs=8))
            pk_pool = ctx.enter_context(tc.tile_pool(name="pk", bufs=8))
            cp_pool = ctx.enter_context(tc.tile_pool(name="cp", bufs=4))
            cv_psum = ctx.enter_context(
                tc.tile_pool(name="cv", bufs=3, space="PSUM"))
            t_psum = ctx.enter_context(
                tc.tile_pool(name="tp", bufs=2, space="PSUM"))

            gt_tiles = {}
            pk_tiles = {}

            def fetch_group(g):
                if g in gt_tiles or g >= ngrp:
                    return
                gt = gt_pool.tile([128, TW], fp8, tag="gt", name=f"gt{g}")
                nc.sync.dma_start(out=gt[:, :], in_=gt_d[g, :, :])
                pk = pk_pool.tile([112, TW], fp8, tag="pk", name=f"pk{g}")
                nc.sync.dma_start(out=pk[:, :], in_=pk_d[g, :, :])
                gt_tiles[g] = gt
                pk_tiles[g] = pk

            # conv inputs first so the PE can start ASAP
            nc.sync.dma_start(out=wall_sb[:, :, :, :], in_=wall_d[:, :])
            for _g in range(5):
                fetch_group(_g)
            nc.sync.dma_start(out=dwt_sb[:, :], in_=dwt_d[:, :])
            nc.sync.dma_start(out=actb_sb[:, :], in_=actb_d[:, :])
            nc.sync.dma_start(out=dbias_sb[:, :], in_=dbias_d[:, :])
            for c in range(nchunk):
                nc.sync.dma_start(out=snorm_sb[c][:, :],
                                  in_=snorm_d[c * 128:(c + 1) * 128, :])
            nc.vector.memset(ones_sb[:, :], 1.0)

            # ---- broadcast the block deltas across partitions via DMA ----
            dview = delta_sb[:, 0, 0]
            dflat = bass.AP(tensor=dview.tensor, offset=dview.offset,
                            ap=[dview.ap[0], [1, 2 * nblk]])
            nc.sync.dma_start(
                out=dflat,
                in_=drow_d[:, 0:2 * nblk].to_broadcast((128, 2 * nblk)))

            BATCH = 3
            lgs_done = [False] * nchunk

            def stage2_group(g):
                """Phase maxima for group g's 32 sentences (DVE)."""
                sg0 = 8 * g
                sgn = min(8, nsg - sg0)
                b0 = sg0 * BPSG         # first block index
                nb = sgn * BPSG         # blocks in this group
                s0 = SPSG * sg0         # first sentence
                nsent = SPSG * sgn
                for fc in range(2):
                    fw = FCH[fc][1]
                    bsl = ball[fc][0:fw, b0:b0 + nb]
                    bs3 = bass.AP(tensor=bsl.tensor, offset=bsl.offset,
                                  ap=[bsl.ap[0], [SBLK, nsent], [1, SBLK]])
                    nc.vector.tensor_reduce(
                        out=pooled[fc][0:fw, 0, s0:s0 + nsent],
                        in_=bs3, axis=AX.X, op=ALU.max)
                    teng = nc.gpsimd if (GP_ON and fc == 1) else nc.vector
                    for ph in range(2):
                        sc = scr[fc][0:fw, 0:nb]
                        teng.tensor_tensor(
                            out=sc, in0=bsl,
                            in1=delta_sb[0:fw, ph, b0:b0 + nb],
                            op=ALU.add)
                        sc3 = bass.AP(tensor=sc.tensor, offset=sc.offset,
                                      ap=[sc.ap[0], [SBLK, nsent], [1, SBLK]])
                        nc.vector.tensor_reduce(
                            out=pooled[fc][0:fw, 1 + ph, s0:s0 + nsent],
                            in_=sc3, axis=AX.X, op=ALU.max)

            def finish_chunk(c):
                """relu + logits matmul for sentence chunk c (128 sentences)."""
                if lgs_done[c]:
                    return
                lgs_done[c] = True
                cs = min(128, ns_pad - 128 * c)
                for fc in range(2):
                    fw = FCH[fc][1]
                    nc.scalar.activation(
                        out=pr[fc][0:fw, :, 128 * c:128 * c + cs],
                        in_=pooled[fc][0:fw, :, 128 * c:128 * c + cs],
                        func=AF.Relu, bias=actb_sb[0:fw, fc:fc + 1], scale=1.0)
                lg_ps = t_psum.tile([128, NREL], f32, tag="tp", name=f"lg{c}")
                nmm = 0
                for j in range(3):
                    for fc, (f0, fw, fwp) in enumerate(FCH):
                        nc.tensor.matmul(
                            out=lg_ps[0:cs, :],
                            lhsT=pr[fc][0:fw, j, 128 * c:128 * c + cs],
                            rhs=dwt_sb[0:fw, (j * 2 + fc) * NREL:
                                       (j * 2 + fc + 1) * NREL],
                            start=(nmm == 0), stop=False,
                            skip_group_check=True)
                        nmm += 1
                nc.tensor.matmul(
                    out=lg_ps[0:cs, :],
                    lhsT=ones_sb[0:1, 0:cs],
                    rhs=dbias_sb[0:1, :],
                    start=False, stop=True, skip_group_check=True)
                nc.scalar.copy(out=lgs[c][0:cs, :], in_=lg_ps[0:cs, :])

            kk = 0
            while kk < nsg:
                bn = min(BATCH, nsg - kk)
                # prefetch future fetch tiles
                fetch_group(kk // HG + 5)
                fetch_group(kk // HG + 6)
                ps = []
                slab = cp_pool.tile([128, bn, 2, PSW], f16, tag="cp",
                                    name=f"sl{kk}")
                for i in range(bn):
                    ps.append(cv_psum.tile([128, 2, PSW], f32, tag="cv",
                                           name=f"cv{kk + i}"))
                # conv matmuls, weights-major for LDW amortization
                for fc, (f0, fw, fwp) in enumerate(FCH):
                    for t in range(4):
                        for i in range(bn):
                            g = (kk + i) // HG
                            l = (kk + i) % HG
                            gtt = gt_tiles[g]
                            if t < 3:
                                gb = gtt[:, 0]
                                rhs = bass.AP(
                                    tensor=gb.tensor,
                                    offset=gb.offset + 2 * (SGW * l + t),
                                    ap=[gb.ap[0], [1, 2], [2, SGW]])
                                nc.tensor.matmul(
                                    out=ps[i][0:fwp, fc, 0:SGW],
                                    lhsT=wall_sb[:, t, :, f0:f0 + fwp],
                                    rhs=rhs, start=(t == 0), stop=False,
                                    perf_mode=DR, skip_group_check=True)
                            else:
                                pkt = pk_tiles[g]
                                pb = pkt[0:112, 0]
                                rhsp = bass.AP(
                                    tensor=pb.tensor,
                                    offset=pb.offset + 2 * (SGW * l + 1),
                                    ap=[[pb.ap[0][0], 112], [1, 2], [2, SGW]])
                                nc.tensor.matmul(
                                    out=ps[i][0:fwp, fc, 0:SGW],
                                    lhsT=wall_sb[0:112, 3, :, f0:f0 + fwp],
                                    rhs=rhsp, start=False, stop=True,
                                    perf_mode=DR, skip_group_check=True)
                # PSUM -> SBUF f16 slab (scalar engine), both fc at once
                for i in range(bn):
                    nc.scalar.copy(out=slab[:, i, :, 0:SGW],
                                   in_=ps[i][:, :, 0:SGW])
                # stage 1: block maxes (DVE) as a 3-level pairwise-max tree;
                # block-half pairing keeps innermost runs contiguous so the
                # first two levels hit the 2x_1p DVE mode.
                for fc, (f0, fw, fwp) in enumerate(FCH):
                    sl = slab[0:fw, 0, 0, 0]
                    pp = sl.ap[0]
                    o = sl.offset + PSW * fc
                    t1 = tmp1[0:fw, 0]
                    t2 = tmp2[0:fw, 0]
                    p1 = t1.ap[0]
                    p2 = t2.ap[0]
                    nc.vector.tensor_tensor(
                        out=bass.AP(tensor=t1.tensor, offset=t1.offset,
                                    ap=[p1, [204, bn], [4, BPSG], [1, 4]]),
                        in0=bass.AP(tensor=sl.tensor, offset=o,
                                    ap=[pp, [2 * PSW, bn], [8, BPSG], [1, 4]]),
                        in1=bass.AP(tensor=sl.tensor, offset=o + 4,
                                    ap=[pp, [2 * PSW, bn], [8, BPSG], [1, 4]]),
                        op=ALU.max)
                    nc.vector.tensor_tensor(
                        out=bass.AP(tensor=t2.tensor, offset=t2.offset,
                                    ap=[p2, [102, bn], [2, BPSG], [1, 2]]),
                        in0=bass.AP(tensor=t1.tensor, offset=t1.offset,
                                    ap=[p1, [204, bn], [4, BPSG], [1, 2]]),
                        in1=bass.AP(tensor=t1.tensor, offset=t1.offset + 2,
                                    ap=[p1, [204, bn], [4, BPSG], [1, 2]]),
                        op=ALU.max)
                    bo = ball[fc][0:fw, kk * BPSG]
                    nc.vector.tensor_tensor(
                        out=bass.AP(tensor=bo.tensor, offset=bo.offset,
                                    ap=[bo.ap[0], [BPSG, bn], [1, BPSG]]),
                        in0=bass.AP(tensor=t2.tensor, offset=t2.offset,
                                    ap=[p2, [102, bn], [2, BPSG]]),
                        in1=bass.AP(tensor=t2.tensor, offset=t2.offset + 1,
                                    ap=[p2, [102, bn], [2, BPSG]]),
                        op=ALU.max)
                kk += bn
                # group boundary: run stage 2 for completed groups
                gdone = kk // 8
                gprev = (kk - bn) // 8
                for g in range(gprev, min(gdone, ngrp)):
                    if 8 * (g + 1) <= kk or kk == nsg:
                        stage2_group(g)
                        # sentence chunks fully covered by finished groups
                        sdone = min(8 * (g + 1), nsg) * SPSG
                        for c in range(nchunk):
                            if (c + 1) * 128 <= sdone:
                                finish_chunk(c)
            if nsg % 8 != 0:
                stage2_group(nsg // 8)
            for c in range(nchunk):
                finish_chunk(c)

            # ---- bag mean + softmax ----
            bg_ps = t_psum.tile([128, NREL], f32, tag="tp", name="bg")
            for c in range(nchunk):
                cs = min(128, ns_pad - 128 * c)
                nc.tensor.matmul(
                    out=bg_ps[0:bags_cap, :],
                    lhsT=snorm_sb[c][0:cs, :],
                    rhs=lgs[c][0:cs, :],
                    start=(c == 0), stop=(c == nchunk - 1),
                    skip_group_check=True)

            t = singles.tile([128, NREL], f32, name="sm")
            nc.vector.tensor_copy(out=t[0:bags_cap, :],
                                  in_=bg_ps[0:bags_cap, :])
            nmax = singles.tile([128, 1], f32, name="nmax")
            nc.vector.reduce_max(out=nmax[0:bags_cap, :], in_=t[0:bags_cap, :],
                                 axis=AX.X, negate=True)
            ex = singles.tile([128, NREL], f32, name="ex")
            nc.scalar.activation(out=ex[0:bags_cap, :], in_=t[0:bags_cap, :],
                                 func=AF.Exp, bias=nmax[0:bags_cap, :],
                                 scale=1.0)
            ssum = singles.tile([128, 1], f32, name="ssum")
            nc.vector.reduce_sum(out=ssum[0:bags_cap, :],
                                 in_=ex[0:bags_cap, :], axis=AX.X)
            rcp = singles.tile([128, 1], f32, name="rcp")
            nc.vector.reciprocal(out=rcp[0:bags_cap, :],
                                 in_=ssum[0:bags_cap, :])
            res = singles.tile([128, NREL], f32, name="res")
            nc.vector.tensor_scalar_mul(res[0:bags_cap, :],
                                        ex[0:bags_cap, :],
                                        rcp[0:bags_cap, :])
            nc.sync.dma_start(out=out_d[:, :], in_=res[0:bags_cap, :])

    nc.compile()
    return nc


def _pad_edge(a):
    return np.concatenate([a[:, :1], a, a[:, -1:]], axis=1)


def kernel(**inputs):
    global LAST_RESULT
    sentences = np.asarray(inputs["sentences"]).astype(np.int64)
    pos1 = np.asarray(inputs["pos1"]).astype(np.int64)
    pos2 = np.asarray(inputs["pos2"]).astype(np.int64)
    masks = np.asarray(inputs["masks"]).astype(np.float32)
    bag_ids = np.asarray(inputs["bag_ids"]).astype(np.int64)
    word_emb = np.asarray(inputs["word_emb"]).astype(np.float32)
    pf1_emb = np.asarray(inputs["pf1_emb"]).astype(np.float32)
    pf2_emb = np.asarray(inputs["pf2_emb"]).astype(np.float32)
    conv_w = np.asarray(inputs["conv_w"]).astype(np.float32)
    conv_b = np.asarray(inputs["conv_b"]).astype(np.float32)
    dense_w = np.asarray(inputs["dense_w"]).astype(np.float32)
    dense_b = np.asarray(inputs["dense_b"]).astype(np.float32)

    # ---- balanced bag-boundary sharding ----
    counts = np.bincount(bag_ids, minlength=NBAGS)
    cum = np.concatenate([[0], np.cumsum(counts)])
    B = [0]
    for r in range(1, NCORES):
        B.append(int(np.argmin(np.abs(cum - N * r // NCORES))))
    B.append(NBAGS)
    for r in range(1, NCORES + 1):
        B[r] = max(B[r], B[r - 1])
    S = [int(cum[b]) for b in B]
    cnt = [S[r + 1] - S[r] for r in range(NCORES)]
    ncap = max(max(cnt), 1)
    nsg = (ncap + SPSG - 1) // SPSG
    ns_pad = SPSG * nsg
    ngrp = (nsg + HG - 1) // HG
    bags_cap = max(B[r + 1] - B[r] for r in range(NCORES))
    nchunk = (ns_pad + 127) // 128
    nblk = ns_pad * SBLK
    dcols = 2 * nblk
    nd = (dcols + 511) // 512
    TW = 2 * (HGW + 2)

    key = (nsg, ngrp, bags_cap, nchunk)
    if key not in _PROGRAM_CACHE:
        _PROGRAM_CACHE[key] = _build_program(nsg, ngrp, bags_cap, nchunk)
    nc = _PROGRAM_CACHE[key]

    # ---- shared parameter prep ----
    e8 = word_emb.astype(FP8)                    # [V, 300]
    e8main = np.ascontiguousarray(e8[:, :256])   # [V, 256]
    e8left = np.zeros((VOCAB, 44), FP8)
    e8left[:, :] = e8[:, 256:300]
    pf1_8 = pf1_emb.astype(FP8)                  # [240, 5]
    pf2_8 = pf2_emb.astype(FP8)

    wdr = np.zeros((3, 128, 2, 240), np.float32)
    for t in range(3):
        for i in range(2):
            wdr[t, :, i, :NF] = conv_w[:, i:256:2, t].T

    wp = np.zeros((112, 2, 240), np.float32)
    for t in range(3):
        for i in range(2):
            wp[32 * t:32 * t + 22, i, :NF] = conv_w[:, 256 + i:300:2, t].T
            wp[96 + 5 * t:96 + 5 * t + 5, i, :NF] = conv_w[:, 300 + i:310:2, t].T
    wp[111, 0, :NF] = 1.0  # mask channel rides the center tap
    wall = np.zeros((128, 4, 480), np.float32)
    wall[:, 0:3] = wdr.transpose(1, 0, 2, 3).reshape(128, 3, 480)
    wall[0:112, 3] = wp.reshape(112, 480)
    wall = wall.astype(FP8).reshape(128, 4 * 480)

    dwt = np.zeros((128, 6 * NREL), np.float32)
    for j in range(3):
        for fc, (f0, fw, fwp) in enumerate(FCH):
            dwt[:fw, (j * 2 + fc) * NREL:(j * 2 + fc + 1) * NREL] = \
                dense_w[:, j * NF + f0:j * NF + f0 + fw].T
    dwt = dwt.astype(F16)

    actb = np.zeros((128, 2), np.float32)
    for fc, (f0, fw, fwp) in enumerate(FCH):
        actb[:fw, fc] = conv_b[f0:f0 + fw]

    dbias = dense_b.reshape(1, NREL).astype(F16)
    fcounts = np.maximum(counts.astype(np.float32), 1.0)

    piece_all = masks.argmax(axis=1)                      # [N, 120]
    lo_all = (piece_all >= 1).argmax(axis=1)
    hi_all = (piece_all >= 2).argmax(axis=1)

    in_maps = []
    for r in range(NCORES):
        s0r, s1r = S[r], S[r + 1]
        nreal = s1r - s0r
        sent = np.zeros((ns_pad, L), np.int64)
        sent[:nreal] = sentences[s0r:s1r]
        p1 = np.zeros((ns_pad, L), np.int64)
        p1[:nreal] = pos1[s0r:s1r]
        p2 = np.zeros((ns_pad, L), np.int64)
        p2[:nreal] = pos2[s0r:s1r]

        sp = _pad_edge(sent)    # [ns_pad, 122]
        p1p = _pad_edge(p1)
        p2p = _pad_edge(p2)

        srcpos = np.zeros((ns_pad, SLOTS), np.int64)
        slotmask = np.full((ns_pad, SLOTS), -2 * MB, np.float32)
        bp = np.zeros((ns_pad, SBLK), np.int64)
        for i in range(nreal):
            a, b_, c_ = _sentence_layout(int(lo_all[s0r + i]),
                                         int(hi_all[s0r + i]))
            srcpos[i], slotmask[i], bp[i] = a, b_, c_

        tok = np.take_along_axis(sp, srcpos, axis=1)      # [ns_pad, 128]
        p1s = np.take_along_axis(p1p, srcpos, axis=1)
        p2s = np.take_along_axis(p2p, srcpos, axis=1)
        # cross-sentence fixup: last trailing slot carries the next
        # sentence's left-edge column
        tok[:-1, -1] = sp[1:, 0]
        p1s[:-1, -1] = p1p[1:, 0]
        p2s[:-1, -1] = p2p[1:, 0]

        Stot = ns_pad * SLOTS
        ghal_t = np.empty(Stot + 4, np.int64)
        ghal_t[2:-2] = tok.reshape(-1)
        ghal_t[:2] = sp[0, 0]
        ghal_t[-2:] = ghal_t[-3]
        ghal_1 = np.empty(Stot + 4, np.int64)
        ghal_1[2:-2] = p1s.reshape(-1)
        ghal_1[:2] = p1p[0, 0]
        ghal_1[-2:] = ghal_1[-3]
        ghal_2 = np.empty(Stot + 4, np.int64)
        ghal_2[2:-2] = p2s.reshape(-1)
        ghal_2[:2] = p2p[0, 0]
        ghal_2[-2:] = ghal_2[-3]
        ghal_m = np.full(Stot + 4, -2 * MB, np.float32)
        ghal_m[2:-2] = slotmask.reshape(-1)
        m8 = ghal_m.astype(FP8).view(np.uint8)

        gt8 = np.zeros((ngrp, 128, TW), np.uint8)
        pk8 = np.zeros((ngrp, 112, TW), np.uint8)
        for g in range(ngrp):
            u0 = HGW * g + 1            # ghal index of tile u=0 (slot -1)
            idx = np.arange(u0, u0 + HGW + 2)
            idx = np.minimum(idx, Stot + 3)
            arr = e8main[ghal_t[idx]].view(np.uint16)       # [4098, 128]
            gt8[g] = np.ascontiguousarray(arr.T).view(np.uint8).reshape(
                128, TW)
            for t in range(3):
                it = np.clip(idx + (t - 1), 0, Stot + 3)
                lv = e8left[ghal_t[it]].view(np.uint16)     # [4098, 22]
                pk8[g, 32 * t:32 * t + 22] = np.ascontiguousarray(
                    lv.T).view(np.uint8).reshape(22, TW)
                pfv = np.concatenate(
                    [pf1_8[ghal_1[it]], pf2_8[ghal_2[it]]],
                    axis=1).view(np.uint16)                  # [4098, 5]
                pk8[g, 96 + 5 * t:96 + 5 * t + 5] = np.ascontiguousarray(
                    pfv.T).view(np.uint8).reshape(5, TW)
            pk8[g, 111, 0::2] = m8[idx]
        gt8 = gt8.view(FP8)
        pk8 = pk8.view(FP8)

        # block deltas: d1 then d2, fp8 row
        drow = np.zeros((1, nd * 512), np.float32)
        d1 = np.where(bp == 1, MB, np.where(bp == 0, -MB, 0.0))
        d2 = np.where(bp == 2, MB, np.where(bp == 0, -MB, 0.0))
        drow[0, :nblk] = d1.reshape(-1)
        drow[0, nblk:2 * nblk] = d2.reshape(-1)
        drow = drow.astype(F16)

        snorm = np.zeros((nchunk * 128, bags_cap), np.float32)
        bags = bag_ids[s0r:s1r]
        snorm[np.arange(nreal), bags - B[r]] = 1.0 / fcounts[bags]
        snorm = snorm.astype(F16)

        in_maps.append({
            "gt8": gt8,
            "pk8": pk8,
            "wall": wall,
            "drow": drow,
            "dwt": dwt,
            "actb": actb,
            "dbias": dbias,
            "snorm": snorm,
        })

    from concourse.bass_utils import run_bass_kernel_spmd

    trace = bool(int(os.environ.get("KERNEL_TRACE", "0")))
    res = run_bass_kernel_spmd(
        nc, in_maps, core_ids=list(range(NCORES)), trace=trace
    )
    LAST_RESULT = res

    out = np.zeros((NBAGS, NREL), np.float32)
    for r in range(NCORES):
        nb = B[r + 1] - B[r]
        if nb > 0:
            out[B[r]:B[r + 1]] = res.results[r]["out"][:nb].astype(np.float32)
    return out


if __name__ == "__main__":
    d = np.load("/root/problem/ref_inputs.npz")
    out = kernel(**{k: d[k] for k in d.files})
    print("out", out.shape, out.dtype)


# revision 7
# speedup vs baseline: 1.0579x; 1.0024x over previous
"""Trainium2 Bass kernel v3 for the PCNN bag-classification model.

Design:
  - Balanced bag-boundary sharding over 8 cores (no collectives).
  - Host ships the full fp8 DR-interleaved conv input stream (no on-device
    gather): channels 0..255 in the main stream tile, the 44 leftover word
    channels + 10 positional channels (pre-shifted per tap) + the mask
    channel in a packed tile.
  - Block-aligned piece layout: each sentence occupies 128 slots = 32 blocks
    of 4; the three PCNN pieces are padded to block boundaries.  Pad slots
    are killed by a conv mask channel (-8); real piece1/2 slots carry -4 so
    block maxes of foreign pieces always lose.
  - conv1d(k=3) as 4 DoubleRow fp8 matmuls per (subgroup=4 sentences,
    filter-chunk), weights batched across 3 subgroups to amortize LDWEIGHTS.
  - Hierarchical max-pool: scalar engine copies PSUM->SBUF f16 slabs, DVE
    reduces blocks of 4 (stage 1) into a per-core block-max array; per group
    of 8 subgroups, DVE computes the 3 phase maxima over block maxes with
    {-4,0,+4} block deltas (stage 2, 8x less data than slot level).
  - Block deltas are broadcast across partitions with a ones-matmul on the
    PE (instead of a 128x DMA broadcast).
  - Dense + bag-mean (segment mean as matmul with per-bag 1/count weights) +
    softmax on-chip, pipelined per 128-sentence chunk.
"""

import os
import sys

for _p in ("/opt/trn_rl_repo",):
    if _p not in sys.path:
        sys.path.insert(0, _p)

import numpy as np
import ml_dtypes

# ---------------- problem constants ----------------
N = 2048
L = 120
NCORES = 8
NF = 230
NREL = 53
NBAGS = 256
VOCAB = 100000
WD = 300
PD = 5

BLK = 8              # slots per block
SBLK = 17            # blocks per sentence
SLOTS = BLK * SBLK   # 136 slots per sentence
SPSG = 3             # sentences per subgroup (PSUM bank: 408 <= 512 f32)
SGW = SPSG * SLOTS   # 408 slots per subgroup
BPSG = SPSG * SBLK   # 51 blocks per subgroup
GRPW = 8 * SGW       # 3264 slots per group (8 subgroups)
HG = 2               # subgroups per fetch tile
HGW = HG * SGW       # 816 slots per fetch tile
PSW = 512            # PSUM tile free width (per filter chunk)
MB = 4.0
FCH = [(0, 128, 128), (128, 102, 112)]  # (f0, fw_real, fw_pad)

FP8 = ml_dtypes.float8_e4m3
F16 = np.float16

_PROGRAM_CACHE = {}
_LAYOUT_CACHE = {}
LAST_RESULT = None


# ---------------- per-sentence block layout ----------------
def _sentence_layout(lo, hi):
    """Returns (srcpos[128] int64 into the 122-wide edge-padded arrays,
    slotmask[128] float {0,-MB,-2MB}, blockpiece[32] int64)."""
    key = (lo, hi)
    hit = _LAYOUT_CACHE.get(key)
    if hit is not None:
        return hit
    lens = [lo, hi - lo, L - hi]
    starts = [0, lo, hi]
    B0 = -(-lens[0] // BLK)
    B1 = -(-lens[1] // BLK)
    B2 = SBLK - B0 - B1
    assert B2 * BLK >= lens[2], (lo, hi)
    Bs = [B0, B1, B2]
    p = [Bs[i] * BLK - lens[i] for i in range(3)]
    sol = None
    for f0 in range(p[0] + 1):
        for f1 in range(p[1] + 1):
            for f2 in range(p[2] + 1):
                b0, b1, b2 = p[0] - f0, p[1] - f1, p[2] - f2
                if (b0 + f1) != 1 and (b1 + f2) != 1 and b2 >= 2:
                    sol = (f0, f1, f2)
                    break
            if sol:
                break
        if sol:
            break
    assert sol is not None, (lens, p)
    f = sol
    srcpos = np.zeros(SLOTS, np.int64)
    slotmask = np.full(SLOTS, -2 * MB, np.float32)
    blockpiece = np.zeros(SBLK, np.int64)
    s = 0
    bidx = 0
    rs, re = [], []
    for i in range(3):
        a, ln = starts[i], lens[i]
        blockpiece[bidx:bidx + Bs[i]] = i
        bidx += Bs[i]
        s += f[i]
        rs.append(s)
        srcpos[s:s + ln] = np.arange(a + 1, a + ln + 1)
        slotmask[s:s + ln] = 0.0 if i == 0 else -MB
        s += ln
        re.append(s)
        s += p[i] - f[i]
    assert s == SLOTS
    srcpos[0:rs[0]] = 0
    for i in range(2):
        r0, r1 = re[i], rs[i + 1]
        if r1 > r0:
            srcpos[r0:r1] = starts[i] + lens[i] + 1
            srcpos[r1 - 1] = starts[i + 1]
    srcpos[re[2]:SLOTS] = L + 1
    out = (srcpos, slotmask, blockpiece)
    _LAYOUT_CACHE[key] = out
    return out


# ---------------- device program ----------------
def _build_program(nsg, ngrp, bags_cap, nchunk):
    import concourse.bass as bass
    import concourse.mybir as mybir
    import concourse.tile as tile
    from concourse import bacc

    f32 = mybir.dt.float32
    f16 = mybir.dt.float16
    fp8 = mybir.dt.float8e4
    AF = mybir.ActivationFunctionType
    AX = mybir.AxisListType
    ALU = mybir.AluOpType
    DR = mybir.MatmulPerfMode.DoubleRow

    ns_pad = SPSG * nsg
    nblk = ns_pad * SBLK            # total blocks per core
    dcols = 2 * nblk                # delta row columns (2 phases)
    nd = (dcols + 511) // 512       # delta broadcast chunks
    TW = 2 * (HGW + 2)              # stream tile bytes per partition
    GP_ON = bool(int(os.environ.get("KERNEL_GP", "1")))

    nc = bacc.Bacc(
        "TRN2", target_bir_lowering=False, debug=False, num_devices=NCORES,
        num_swdge_queues=1,
    )

    gt_d = nc.dram_tensor("gt8", [ngrp, 128, TW], fp8, kind="ExternalInput").ap()
    pk_d = nc.dram_tensor("pk8", [ngrp, 112, TW], fp8, kind="ExternalInput").ap()
    wall_d = nc.dram_tensor("wall", [128, 4 * 480], fp8,
                            kind="ExternalInput").ap()
    drow_d = nc.dram_tensor("drow", [1, nd * 512], f16,
                            kind="ExternalInput").ap()
    dwt_d = nc.dram_tensor("dwt", [128, 6 * NREL], f16,
                           kind="ExternalInput").ap()
    actb_d = nc.dram_tensor("actb", [128, 2], f32, kind="ExternalInput").ap()
    dbias_d = nc.dram_tensor("dbias", [1, NREL], f16, kind="ExternalInput").ap()
    snorm_d = nc.dram_tensor("snorm", [nchunk * 128, bags_cap], f16,
                             kind="ExternalInput").ap()
    out_d = nc.dram_tensor("out", [bags_cap, NREL], f32,
                           kind="ExternalOutput").ap()

    with tile.TileContext(nc) as tc:
        import contextlib

        ctx = contextlib.ExitStack()
        with ctx:
            singles = ctx.enter_context(tc.tile_pool(name="singles", bufs=1))

            wall_sb = singles.tile([128, 4, 2, 240], fp8, name="wall")
            dwt_sb = singles.tile([128, 6 * NREL], f16)
            actb_sb = singles.tile([128, 2], f32)
            dbias_sb = singles.tile([1, NREL], f16)
            snorm_sb = [singles.tile([128, bags_cap], f16, name=f"sn{c}")
                        for c in range(nchunk)]
            ones_sb = singles.tile([1, 128], f16)
            ball = [singles.tile([128, nblk], f16, name=f"ball{c}")
                    for c in range(2)]
            tmp1 = singles.tile([128, 3 * 204], f16, name="tmp1")
            tmp2 = singles.tile([128, 3 * 102], f16, name="tmp2")
            delta_sb = singles.tile([128, 2, nblk], f16)
            scr = [singles.tile([128, 8 * BPSG], f16, name=f"scr{c}")
                   for c in range(2)]
            gtmp = singles.tile([128, 3, 256], f32, name="gtmp")
            pooled = [singles.tile([128, 3, ns_pad], f16, name=f"pool{c}")
                      for c in range(2)]
            pr = [singles.tile([128, 3, ns_pad], f16, name=f"pr{c}")
                  for c in range(2)]
            lgs = [singles.tile([128, NREL], f16, name=f"lgs{c}")
                   for c in range(nchunk)]

            gt_pool = ctx.enter_context(tc.tile_pool(name="gt", bufs=10))
            pk_pool = ctx.enter_context(tc.tile_pool(name="pk", bufs=10))
            cp_pool = ctx.enter_context(tc.tile_pool(name="cp", bufs=4))
            cv_psum = ctx.enter_context(
                tc.tile_pool(name="cv", bufs=3, space="PSUM"))
            t_psum = ctx.enter_context(
                tc.tile_pool(name="tp", bufs=2, space="PSUM"))

            gt_tiles = {}
            pk_tiles = {}

            def fetch_group(g):
                if g in gt_tiles or g >= ngrp:
                    return
                gt = gt_pool.tile([128, TW], fp8, tag="gt", name=f"gt{g}")
                nc.sync.dma_start(out=gt[:, :], in_=gt_d[g, :, :])
                pk = pk_pool.tile([112, TW], fp8, tag="pk", name=f"pk{g}")
                nc.sync.dma_start(out=pk[:, :], in_=pk_d[g, :, :])
                gt_tiles[g] = gt
                pk_tiles[g] = pk

            # conv inputs first so the PE can start ASAP
            nc.sync.dma_start(out=wall_sb[:, :, :, :], in_=wall_d[:, :])
            for _g in range(8):
                fetch_group(_g)
            nc.sync.dma_start(out=dwt_sb[:, :], in_=dwt_d[:, :])
            nc.sync.dma_start(out=actb_sb[:, :], in_=actb_d[:, :])
            nc.sync.dma_start(out=dbias_sb[:, :], in_=dbias_d[:, :])
            for c in range(nchunk):
                nc.sync.dma_start(out=snorm_sb[c][:, :],
                                  in_=snorm_d[c * 128:(c + 1) * 128, :])
            nc.vector.memset(ones_sb[:, :], 1.0)

            # ---- broadcast the block deltas across partitions via DMA ----
            dview = delta_sb[:, 0, 0]
            dflat = bass.AP(tensor=dview.tensor, offset=dview.offset,
                            ap=[dview.ap[0], [1, 2 * nblk]])
            nc.sync.dma_start(
                out=dflat,
                in_=drow_d[:, 0:2 * nblk].to_broadcast((128, 2 * nblk)))

            BATCH = 3
            lgs_done = [False] * nchunk

            def stage2_group(g):
                """Phase maxima for group g's 32 sentences (DVE)."""
                sg0 = 8 * g
                sgn = min(8, nsg - sg0)
                b0 = sg0 * BPSG         # first block index
                nb = sgn * BPSG         # blocks in this group
                s0 = SPSG * sg0         # first sentence
                nsent = SPSG * sgn
                for fc in range(2):
                    fw = FCH[fc][1]
                    bsl = ball[fc][0:fw, b0:b0 + nb]
                    bs3 = bass.AP(tensor=bsl.tensor, offset=bsl.offset,
                                  ap=[bsl.ap[0], [SBLK, nsent], [1, SBLK]])
                    nc.vector.tensor_reduce(
                        out=pooled[fc][0:fw, 0, s0:s0 + nsent],
                        in_=bs3, axis=AX.X, op=ALU.max)
                    teng = nc.gpsimd if (GP_ON and fc == 1) else nc.vector
                    for ph in range(2):
                        sc = scr[fc][0:fw, 0:nb]
                        teng.tensor_tensor(
                            out=sc, in0=bsl,
                            in1=delta_sb[0:fw, ph, b0:b0 + nb],
                            op=ALU.add)
                        sc3 = bass.AP(tensor=sc.tensor, offset=sc.offset,
                                      ap=[sc.ap[0], [SBLK, nsent], [1, SBLK]])
                        nc.vector.tensor_reduce(
                            out=pooled[fc][0:fw, 1 + ph, s0:s0 + nsent],
                            in_=sc3, axis=AX.X, op=ALU.max)

            def finish_chunk(c):
                """relu + logits matmul for sentence chunk c (128 sentences)."""
                if lgs_done[c]:
                    return
                lgs_done[c] = True
                cs = min(128, ns_pad - 128 * c)
                for fc in range(2):
                    fw = FCH[fc][1]
                    nc.scalar.activation(
                        out=pr[fc][0:fw, :, 128 * c:128 * c + cs],
                        in_=pooled[fc][0:fw, :, 128 * c:128 * c + cs],
                        func=AF.Relu, bias=actb_sb[0:fw, fc:fc + 1], scale=1.0)
                lg_ps = t_psum.tile([128, NREL], f32, tag="tp", name=f"lg{c}")
                nmm = 0
                for j in range(3):
                    for fc, (f0, fw, fwp) in enumerate(FCH):
                        nc.tensor.matmul(
                            out=lg_ps[0:cs, :],
                            lhsT=pr[fc][0:fw, j, 128 * c:128 * c + cs],
                            rhs=dwt_sb[0:fw, (j * 2 + fc) * NREL:
                                       (j * 2 + fc + 1) * NREL],
                            start=(nmm == 0), stop=False,
                            skip_group_check=True)
                        nmm += 1
                nc.tensor.matmul(
                    out=lg_ps[0:cs, :],
                    lhsT=ones_sb[0:1, 0:cs],
                    rhs=dbias_sb[0:1, :],
                    start=False, stop=True, skip_group_check=True)
                nc.scalar.copy(out=lgs[c][0:cs, :], in_=lg_ps[0:cs, :])

            kk = 0
            while kk < nsg:
                bn = min(BATCH, nsg - kk)
                # prefetch future fetch tiles
                fetch_group(kk // HG + 7)
                fetch_group(kk // HG + 8)
                ps = []
                slab = cp_pool.tile([128, bn, 2, PSW], f16, tag="cp",
                                    name=f"sl{kk}")
                for i in range(bn):
                    ps.append(cv_psum.tile([128, 2, PSW], f32, tag="cv",
                                           name=f"cv{kk + i}"))
                # conv matmuls, weights-major for LDW amortization
                for fc, (f0, fw, fwp) in enumerate(FCH):
                    for t in range(4):
                        for i in range(bn):
                            g = (kk + i) // HG
                            l = (kk + i) % HG
                            gtt = gt_tiles[g]
                            if t < 3:
                                gb = gtt[:, 0]
                                rhs = bass.AP(
                                    tensor=gb.tensor,
                                    offset=gb.offset + 2 * (SGW * l + t),
                                    ap=[gb.ap[0], [1, 2], [2, SGW]])
                                nc.tensor.matmul(
                                    out=ps[i][0:fwp, fc, 0:SGW],
                                    lhsT=wall_sb[:, t, :, f0:f0 + fwp],
                                    rhs=rhs, start=(t == 0), stop=False,
                                    perf_mode=DR, skip_group_check=True)
                            else:
                                pkt = pk_tiles[g]
                                pb = pkt[0:112, 0]
                                rhsp = bass.AP(
                                    tensor=pb.tensor,
                                    offset=pb.offset + 2 * (SGW * l + 1),
                                    ap=[[pb.ap[0][0], 112], [1, 2], [2, SGW]])
                                nc.tensor.matmul(
                                    out=ps[i][0:fwp, fc, 0:SGW],
                                    lhsT=wall_sb[0:112, 3, :, f0:f0 + fwp],
                                    rhs=rhsp, start=False, stop=True,
                                    perf_mode=DR, skip_group_check=True)
                # PSUM -> SBUF f16 slab (scalar engine), both fc at once
                for i in range(bn):
                    nc.scalar.copy(out=slab[:, i, :, 0:SGW],
                                   in_=ps[i][:, :, 0:SGW])
                # stage 1: block maxes (DVE) as a 3-level pairwise-max tree;
                # block-half pairing keeps innermost runs contiguous so the
                # first two levels hit the 2x_1p DVE mode.
                for fc, (f0, fw, fwp) in enumerate(FCH):
                    sl = slab[0:fw, 0, 0, 0]
                    pp = sl.ap[0]
                    o = sl.offset + PSW * fc
                    t1 = tmp1[0:fw, 0]
                    t2 = tmp2[0:fw, 0]
                    p1 = t1.ap[0]
                    p2 = t2.ap[0]
                    nc.vector.tensor_tensor(
                        out=bass.AP(tensor=t1.tensor, offset=t1.offset,
                                    ap=[p1, [204, bn], [4, BPSG], [1, 4]]),
                        in0=bass.AP(tensor=sl.tensor, offset=o,
                                    ap=[pp, [2 * PSW, bn], [8, BPSG], [1, 4]]),
                        in1=bass.AP(tensor=sl.tensor, offset=o + 4,
                                    ap=[pp, [2 * PSW, bn], [8, BPSG], [1, 4]]),
                        op=ALU.max)
                    nc.vector.tensor_tensor(
                        out=bass.AP(tensor=t2.tensor, offset=t2.offset,
                                    ap=[p2, [102, bn], [2, BPSG], [1, 2]]),
                        in0=bass.AP(tensor=t1.tensor, offset=t1.offset,
                                    ap=[p1, [204, bn], [4, BPSG], [1, 2]]),
                        in1=bass.AP(tensor=t1.tensor, offset=t1.offset + 2,
                                    ap=[p1, [204, bn], [4, BPSG], [1, 2]]),
                        op=ALU.max)
                    bo = ball[fc][0:fw, kk * BPSG]
                    nc.vector.tensor_tensor(
                        out=bass.AP(tensor=bo.tensor, offset=bo.offset,
                                    ap=[bo.ap[0], [BPSG, bn], [1, BPSG]]),
                        in0=bass.AP(tensor=t2.tensor, offset=t2.offset,
                                    ap=[p2, [102, bn], [2, BPSG]]),
                        in1=bass.AP(tensor=t2.tensor, offset=t2.offset + 1,
                                    ap=[p2, [102, bn], [2, BPSG]]),
                        op=ALU.max)
                kk += bn
                # group boundary: run stage 2 for completed groups
                gdone = kk // 8
                gprev = (kk - bn) // 8
                for g in range(gprev, min(gdone, ngrp)):
                    if 8 * (g + 1) <= kk or kk == nsg:
                        stage2_group(g)
                        # sentence chunks fully covered by finished groups
                        sdone = min(8 * (g + 1), nsg) * SPSG
                        for c in range(nchunk):
                            if (c + 1) * 128 <= sdone:
                                finish_chunk(c)
            if nsg % 8 != 0:
                stage2_group(nsg // 8)
            for c in range(nchunk):
                finish_chunk(c)

            # ---- bag mean + softmax ----
            bg_ps = t_psum.tile([128, NREL], f32, tag="tp", name="bg")
            for c in range(nchunk):
                cs = min(128, ns_pad - 128 * c)
                nc.tensor.matmul(
                    out=bg_ps[0:bags_cap, :],
                    lhsT=snorm_sb[c][0:cs, :],
                    rhs=lgs[c][0:cs, :],
                    start=(c == 0), stop=(c == nchunk - 1),
                    skip_group_check=True)

            t = singles.tile([128, NREL], f32, name="sm")
            nc.vector.tensor_copy(out=t[0:bags_cap, :],
                                  in_=bg_ps[0:bags_cap, :])
            nmax = singles.tile([128, 1], f32, name="nmax")
            nc.vector.reduce_max(out=nmax[0:bags_cap, :], in_=t[0:bags_cap, :],
                                 axis=AX.X, negate=True)
            ex = singles.tile([128, NREL], f32, name="ex")
            nc.scalar.activation(out=ex[0:bags_cap, :], in_=t[0:bags_cap, :],
                                 func=AF.Exp, bias=nmax[0:bags_cap, :],
                                 scale=1.0)
            ssum = singles.tile([128, 1], f32, name="ssum")
            nc.vector.reduce_sum(out=ssum[0:bags_cap, :],
                                 in_=ex[0:bags_cap, :], axis=AX.X)
            rcp = singles.tile([128, 1], f32, name="rcp")
            nc.vector.reciprocal(out=rcp[0:bags_cap, :],
                                 in_=ssum[0:bags_cap, :])
            res = singles.tile([128, NREL], f32, name="res")
            nc.vector.tensor_scalar_mul(res[0:bags_cap, :],
                                        ex[0:bags_cap, :],
                                        rcp[0:bags_cap, :])
            nc.sync.dma_start(out=out_d[:, :], in_=res[0:bags_cap, :])

    nc.compile()
    return nc


def _pad_edge(a):
    return np.concatenate([a[:, :1], a, a[:, -1:]], axis=1)


def kernel(**inputs):
    global LAST_RESULT
    sentences = np.asarray(inputs["sentences"]).astype(np.int64)
    pos1 = np.asarray(inputs["pos1"]).astype(np.int64)
    pos2 = np.asarray(inputs["pos2"]).astype(np.int64)
    masks = np.asarray(inputs["masks"]).astype(np.float32)
    bag_ids = np.asarray(inputs["bag_ids"]).astype(np.int64)
    word_emb = np.asarray(inputs["word_emb"]).astype(np.float32)
    pf1_emb = np.asarray(inputs["pf1_emb"]).astype(np.float32)
    pf2_emb = np.asarray(inputs["pf2_emb"]).astype(np.float32)
    conv_w = np.asarray(inputs["conv_w"]).astype(np.float32)
    conv_b = np.asarray(inputs["conv_b"]).astype(np.float32)
    dense_w = np.asarray(inputs["dense_w"]).astype(np.float32)
    dense_b = np.asarray(inputs["dense_b"]).astype(np.float32)

    # ---- balanced bag-boundary sharding ----
    counts = np.bincount(bag_ids, minlength=NBAGS)
    cum = np.concatenate([[0], np.cumsum(counts)])
    B = [0]
    for r in range(1, NCORES):
        B.append(int(np.argmin(np.abs(cum - N * r // NCORES))))
    B.append(NBAGS)
    for r in range(1, NCORES + 1):
        B[r] = max(B[r], B[r - 1])
    S = [int(cum[b]) for b in B]
    cnt = [S[r + 1] - S[r] for r in range(NCORES)]
    ncap = max(max(cnt), 1)
    nsg = (ncap + SPSG - 1) // SPSG
    ns_pad = SPSG * nsg
    ngrp = (nsg + HG - 1) // HG
    bags_cap = max(B[r + 1] - B[r] for r in range(NCORES))
    nchunk = (ns_pad + 127) // 128
    nblk = ns_pad * SBLK
    dcols = 2 * nblk
    nd = (dcols + 511) // 512
    TW = 2 * (HGW + 2)

    key = (nsg, ngrp, bags_cap, nchunk)
    if key not in _PROGRAM_CACHE:
        _PROGRAM_CACHE[key] = _build_program(nsg, ngrp, bags_cap, nchunk)
    nc = _PROGRAM_CACHE[key]

    # ---- shared parameter prep ----
    e8 = word_emb.astype(FP8)                    # [V, 300]
    e8main = np.ascontiguousarray(e8[:, :256])   # [V, 256]
    e8left = np.zeros((VOCAB, 44), FP8)
    e8left[:, :] = e8[:, 256:300]
    pf1_8 = pf1_emb.astype(FP8)                  # [240, 5]
    pf2_8 = pf2_emb.astype(FP8)

    wdr = np.zeros((3, 128, 2, 240), np.float32)
    for t in range(3):
        for i in range(2):
            wdr[t, :, i, :NF] = conv_w[:, i:256:2, t].T

    wp = np.zeros((112, 2, 240), np.float32)
    for t in range(3):
        for i in range(2):
            wp[32 * t:32 * t + 22, i, :NF] = conv_w[:, 256 + i:300:2, t].T
            wp[96 + 5 * t:96 + 5 * t + 5, i, :NF] = conv_w[:, 300 + i:310:2, t].T
    wp[111, 0, :NF] = 1.0  # mask channel rides the center tap
    wall = np.zeros((128, 4, 480), np.float32)
    wall[:, 0:3] = wdr.transpose(1, 0, 2, 3).reshape(128, 3, 480)
    wall[0:112, 3] = wp.reshape(112, 480)
    wall = wall.astype(FP8).reshape(128, 4 * 480)

    dwt = np.zeros((128, 6 * NREL), np.float32)
    for j in range(3):
        for fc, (f0, fw, fwp) in enumerate(FCH):
            dwt[:fw, (j * 2 + fc) * NREL:(j * 2 + fc + 1) * NREL] = \
                dense_w[:, j * NF + f0:j * NF + f0 + fw].T
    dwt = dwt.astype(F16)

    actb = np.zeros((128, 2), np.float32)
    for fc, (f0, fw, fwp) in enumerate(FCH):
        actb[:fw, fc] = conv_b[f0:f0 + fw]

    dbias = dense_b.reshape(1, NREL).astype(F16)
    fcounts = np.maximum(counts.astype(np.float32), 1.0)

    piece_all = masks.argmax(axis=1)                      # [N, 120]
    lo_all = (piece_all >= 1).argmax(axis=1)
    hi_all = (piece_all >= 2).argmax(axis=1)

    in_maps = []
    for r in range(NCORES):
        s0r, s1r = S[r], S[r + 1]
        nreal = s1r - s0r
        sent = np.zeros((ns_pad, L), np.int64)
        sent[:nreal] = sentences[s0r:s1r]
        p1 = np.zeros((ns_pad, L), np.int64)
        p1[:nreal] = pos1[s0r:s1r]
        p2 = np.zeros((ns_pad, L), np.int64)
        p2[:nreal] = pos2[s0r:s1r]

        sp = _pad_edge(sent)    # [ns_pad, 122]
        p1p = _pad_edge(p1)
        p2p = _pad_edge(p2)

        srcpos = np.zeros((ns_pad, SLOTS), np.int64)
        slotmask = np.full((ns_pad, SLOTS), -2 * MB, np.float32)
        bp = np.zeros((ns_pad, SBLK), np.int64)
        for i in range(nreal):
            a, b_, c_ = _sentence_layout(int(lo_all[s0r + i]),
                                         int(hi_all[s0r + i]))
            srcpos[i], slotmask[i], bp[i] = a, b_, c_

        tok = np.take_along_axis(sp, srcpos, axis=1)      # [ns_pad, 128]
        p1s = np.take_along_axis(p1p, srcpos, axis=1)
        p2s = np.take_along_axis(p2p, srcpos, axis=1)
        # cross-sentence fixup: last trailing slot carries the next
        # sentence's left-edge column
        tok[:-1, -1] = sp[1:, 0]
        p1s[:-1, -1] = p1p[1:, 0]
        p2s[:-1, -1] = p2p[1:, 0]

        Stot = ns_pad * SLOTS
        ghal_t = np.empty(Stot + 4, np.int64)
        ghal_t[2:-2] = tok.reshape(-1)
        ghal_t[:2] = sp[0, 0]
        ghal_t[-2:] = ghal_t[-3]
        ghal_1 = np.empty(Stot + 4, np.int64)
        ghal_1[2:-2] = p1s.reshape(-1)
        ghal_1[:2] = p1p[0, 0]
        ghal_1[-2:] = ghal_1[-3]
        ghal_2 = np.empty(Stot + 4, np.int64)
        ghal_2[2:-2] = p2s.reshape(-1)
        ghal_2[:2] = p2p[0, 0]
        ghal_2[-2:] = ghal_2[-3]
        ghal_m = np.full(Stot + 4, -2 * MB, np.float32)
        ghal_m[2:-2] = slotmask.reshape(-1)
        m8 = ghal_m.astype(FP8).view(np.uint8)

        gt8 = np.zeros((ngrp, 128, TW), np.uint8)
        pk8 = np.zeros((ngrp, 112, TW), np.uint8)
        for g in range(ngrp):
            u0 = HGW * g + 1            # ghal index of tile u=0 (slot -1)
            idx = np.arange(u0, u0 + HGW + 2)
            idx = np.minimum(idx, Stot + 3)
            arr = e8main[ghal_t[idx]].view(np.uint16)       # [4098, 128]
            gt8[g] = np.ascontiguousarray(arr.T).view(np.uint8).reshape(
                128, TW)
            for t in range(3):
                it = np.clip(idx + (t - 1), 0, Stot + 3)
                lv = e8left[ghal_t[it]].view(np.uint16)     # [4098, 22]
                pk8[g, 32 * t:32 * t + 22] = np.ascontiguousarray(
                    lv.T).view(np.uint8).reshape(22, TW)
                pfv = np.concatenate(
                    [pf1_8[ghal_1[it]], pf2_8[ghal_2[it]]],
                    axis=1).view(np.uint16)                  # [4098, 5]
                pk8[g, 96 + 5 * t:96 + 5 * t + 5] = np.ascontiguousarray(
                    pfv.T).view(np.uint8).reshape(5, TW)
            pk8[g, 111, 0::2] = m8[idx]
        gt8 = gt8.view(FP8)
        pk8 = pk8.view(FP8)

        # block deltas: d1 then d2, fp8 row
        drow = np.zeros((1, nd * 512), np.float32)
        d1 = np.where(bp == 1, MB, np.where(bp == 0, -MB, 0.0))
        d2 = np.where(bp == 2, MB, np.where(bp == 0, -MB, 0.0))
        drow[0, :nblk] = d1.reshape(-1)
        drow[0, nblk:2 * nblk] = d2.reshape(-1)
        drow = drow.astype(F16)

        snorm = np.zeros((nchunk * 128, bags_cap), np.float32)
        bags = bag_ids[s0r:s1r]
        snorm[np.arange(nreal), bags - B[r]] = 1.0 / fcounts[bags]
        snorm = snorm.astype(F16)

        in_maps.append({
            "gt8": gt8,
            "pk8": pk8,
            "wall": wall,
            "drow": drow,
            "dwt": dwt,
            "actb": actb,
            "dbias": dbias,
            "snorm": snorm,
        })

    from concourse.bass_utils import run_bass_kernel_spmd

    trace = bool(int(os.environ.get("KERNEL_TRACE", "0")))
    res = run_bass_kernel_spmd(
        nc, in_maps, core_ids=list(range(NCORES)), trace=trace
    )
    LAST_RESULT = res

    out = np.zeros((NBAGS, NREL), np.float32)
    for r in range(NCORES):
        nb = B[r + 1] - B[r]
        if nb > 0:
            out[B[r]:B[r + 1]] = res.results[r]["out"][:nb].astype(np.float32)
    return out


if __name__ == "__main__":
    d = np.load("/root/problem/ref_inputs.npz")
    out = kernel(**{k: d[k] for k in d.files})
    print("out", out.shape, out.dtype)
